# revision 1
# baseline (speedup 1.0000x reference)
"""Trainium2 Bass kernel for nn_CustomAttnProcessor (dense transformer block).

Data-parallel over batch B=8 across 8 NeuronCores; one batch element per core.

Per-core dataflow (channel-major activations: [feature_partition, token_free]):
  xT = concat(hiddenT, obj @ linear_w)            [1280, 1056pad] f32r
  ln1T = LN(xT) -> fp8                            masked self-attention
      QKV via fp8 DoubleRow matmuls; sim/AV in bf16; only the 1024 visual
      query columns are computed (object-query outputs are discarded).
      P^T = exp(simT)*maskT (mask prequantized to bf16); denominator via an
      appended ones-column on V, batched reciprocal via exp(-ln(x)) on ACT,
      broadcast back with a K=2 selector matmul.
  hsT = hiddenT + tanh(a_attn)*attn               (tanh folded into weights)
  hsT += tanh(a_dense)*GEGLU_FFN(LN(hsT))         (fp8 DoubleRow FFN)
  out = cross_attention(LN(hsT), enc)             token-major output
"""

import math
import os
import sys

import numpy as np
import ml_dtypes

sys.path.insert(0, "/opt/trn_rl_repo")

import concourse.bass as bass
import concourse.tile as tile
from concourse import bacc, mybir
from concourse.bass_utils import run_bass_kernel_spmd

F32 = mybir.dt.float32
F32R = mybir.dt.float32r
BF16 = mybir.dt.bfloat16
FP8 = mybir.dt.float8e4
AF = mybir.ActivationFunctionType
ALU = mybir.AluOpType
PM = mybir.MatmulPerfMode
# CoreSim lacks Gelu; Tanh is a stand-in for structural sim debugging only
GELU_AF = AF.Tanh if os.environ.get("SIM_SAFE_GELU") else AF.Gelu

B = 8
NV = 1024          # visual tokens (also the only query columns we compute)
NOBJ = 30
N = NV + NOBJ      # 1054
NP = 1056          # padded token count (keys)
NJC = 9            # key-dim 128-chunks over NP (last chunk = 32 rows)
D = 1280
KD = D // 128      # 10
DTXT = 768
KT = DTXT // 128   # 6
LTXT = 77
LTP = 78           # padded
HC, CC = 8, 64     # masked self-attention heads
HA, CA = 20, 64    # cross-attention heads
INNER_C = HC * CC  # 512
INNER_A = HA * CA  # 1280
DFF = 4 * D        # 5120
KF = DFF // 128    # 40
EPS = 1e-5
SCALE = CC ** -0.5  # 0.125

# fp8 weight quant scales (powers of 2; folded back via activation scales)
SW_QKV = 256.0     # w_q/k/v, w_qa
SW_CO = 2048.0     # w_co (tanh-folded)
SW_G = 16.0        # w_geglu (also leaves ffT scaled x16 for fp8 range)
SW_F = 2048.0      # w_ffout (tanh-folded)
CAT_S = 8.0        # catT = 8*attn (keeps fp8 out of denormals)

IC_NP = [(0, 512), (512, 512), (1024, 32)]   # token chunks for 1056
IC_NV = [(0, 512), (512, 512)]               # token chunks for 1024
DC_D = [(0, 512), (512, 512), (1024, 256)]   # feature chunks for 1280


def _emit_layernorm(tc, nc, x, out, n_tok, kc_n, lidx, normgb, m2, ones_r, eps_t):
    """LN over the partition (feature) axis of channel-major x.

    x:   [128, kc_n, n_tok] f32r; out: [128, kc_n, n_tok] (any dtype)
    normgb: [2, 3, KD, 128] f32r rows (partition0=g, partition1=b)
    m2:  [2, NP] f32r scratch rows (partition1 = -1 const)
    out = x*(g (x) rstd) - (g (x) mu*rstd - b (x) 1)
    """
    chunks = [(o, w) for (o, w) in IC_NP if o < n_tok]
    inv_d = 1.0 / float(kc_n * 128)
    with (
        tc.tile_pool(name="ln_rows", bufs=1) as rows,
        tc.tile_pool(name="ln_sq", bufs=2) as sqp,
        tc.tile_pool(name="ln_t1", bufs=3) as t1p,
    ):
        mu_row = rows.tile([1, n_tok], F32R, tag="mu_row")
        ex_row = rows.tile([1, n_tok], F32, tag="ex_row")
        mu2_row = rows.tile([1, n_tok], F32, tag="mu2_row")
        var_row = rows.tile([1, n_tok], F32, tag="var_row")
        rs_row = rows.tile([1, n_tok], F32R, tag="rs_row")
        with tc.tile_pool(name="ln_stat", bufs=1, space="PSUM") as stat_ps:
            ps_mu = [stat_ps.tile([1, w], F32, tag=f"ps_mu{i}", name=f"ps_mu{i}")
                     for i, (o, w) in enumerate(chunks)]
            ps_ex = [stat_ps.tile([1, w], F32, tag=f"ps_ex{i}", name=f"ps_ex{i}")
                     for i, (o, w) in enumerate(chunks)]
            for kc in range(kc_n):
                sq = sqp.tile([128, n_tok], F32R, tag="ln_sq")
                nc.scalar.activation(sq[:], x[:, kc, :].bitcast(F32), AF.Square)
                for i, (o, w) in enumerate(chunks):
                    nc.tensor.matmul(ps_mu[i][:], ones_r[:], x[:, kc, o:o + w],
                                     start=(kc == 0), stop=(kc == kc_n - 1))
                    nc.tensor.matmul(ps_ex[i][:], ones_r[:], sq[:, o:o + w],
                                     start=(kc == 0), stop=(kc == kc_n - 1))
            for i, (o, w) in enumerate(chunks):
                nc.scalar.activation(mu_row[:, o:o + w], ps_mu[i][:], AF.Copy,
                                     scale=inv_d)
                nc.scalar.activation(ex_row[:, o:o + w], ps_ex[i][:], AF.Copy,
                                     scale=inv_d)
        nc.vector.tensor_mul(mu2_row[:], mu_row[:].bitcast(F32), mu_row[:].bitcast(F32))
        nc.vector.tensor_sub(var_row[:], ex_row[:], mu2_row[:])
        # rstd = exp(-0.5*ln(var+eps)) — avoids the slow 1-lane DVE reciprocal
        nc.scalar.activation(mu2_row[:], var_row[:], AF.Ln, bias=eps_t[:])
        nc.scalar.activation(rs_row[:], mu2_row[:], AF.Exp, scale=-0.5)
        nc.vector.tensor_mul(m2[0:1, 0:n_tok], mu_row[:].bitcast(F32),
                             rs_row[:].bitcast(F32))
        with tc.tile_pool(name="ln_bps", bufs=2, space="PSUM") as bps:
            for kc in range(kc_n):
                for i, (o, w) in enumerate(chunks):
                    b1 = bps.tile([128, 512], F32, tag="ln_b1")
                    b2 = bps.tile([128, 512], F32, tag="ln_b2")
                    nc.tensor.matmul(b1[:, :w], normgb[0:1, lidx, kc, :],
                                     rs_row[:, o:o + w], start=True, stop=True)
                    nc.tensor.matmul(b2[:, :w], normgb[0:2, lidx, kc, :],
                                     m2[0:2, o:o + w], start=True, stop=True)
                    t1 = t1p.tile([128, 512], BF16, tag="ln_t1")
                    nc.vector.tensor_mul(t1[:, :w], x[:, kc, o:o + w].bitcast(F32),
                                         b1[:, :w])
                    nc.vector.tensor_sub(out[:, kc, o:o + w], t1[:, :w], b2[:, :w])


def build_nc():
    nc = bacc.Bacc("TRN2", target_bir_lowering=False, debug=False, num_devices=B)

    # ---- DRAM I/O (per core) ----
    d_hidT = nc.dram_tensor("hidT", [D, NV], F32R, kind="ExternalInput").ap()
    d_objT = nc.dram_tensor("objT", [DTXT, NOBJ], BF16, kind="ExternalInput").ap()
    d_encT = nc.dram_tensor("encT", [DTXT, LTP], F32R, kind="ExternalInput").ap()
    d_mask = nc.dram_tensor("maskTb", [HC, NP, NV], BF16, kind="ExternalInput").ap()
    d_wlin = nc.dram_tensor("w_lin", [DTXT, D], BF16, kind="ExternalInput").ap()
    d_blin = nc.dram_tensor("b_lin", [D], F32, kind="ExternalInput").ap()
    d_wq = nc.dram_tensor("w_q8", [D, INNER_C], FP8, kind="ExternalInput").ap()
    d_wk = nc.dram_tensor("w_k8", [D, INNER_C], FP8, kind="ExternalInput").ap()
    d_wv = nc.dram_tensor("w_v8", [D, INNER_C], FP8, kind="ExternalInput").ap()
    d_wco = nc.dram_tensor("w_co8", [INNER_C, D], FP8, kind="ExternalInput").ap()
    d_bco = nc.dram_tensor("bco_rows", [1, KD, 128], F32R, kind="ExternalInput").ap()
    d_wg = nc.dram_tensor("w_g8", [D, 2 * DFF], FP8, kind="ExternalInput").ap()
    d_bga = nc.dram_tensor("bg_a16", [DFF], F32, kind="ExternalInput").ap()
    d_bgg = nc.dram_tensor("bg_g", [DFF], F32, kind="ExternalInput").ap()
    d_wf = nc.dram_tensor("w_f8", [DFF, D], FP8, kind="ExternalInput").ap()
    d_bf = nc.dram_tensor("bf_rows", [1, KD, 128], F32R, kind="ExternalInput").ap()
    d_wqa = nc.dram_tensor("w_qa8", [D, INNER_A], BF16, kind="ExternalInput").ap()
    d_wka = nc.dram_tensor("w_ka", [DTXT, INNER_A], F32R, kind="ExternalInput").ap()
    d_wva = nc.dram_tensor("w_va", [DTXT, INNER_A], F32R, kind="ExternalInput").ap()
    d_woa = nc.dram_tensor("w_oa", [INNER_A, D], BF16, kind="ExternalInput").ap()
    d_boa = nc.dram_tensor("b_oa", [D], F32, kind="ExternalInput").ap()
    d_normgb = nc.dram_tensor("normgb", [2, 3, KD, 128], F32R,
                              kind="ExternalInput").ap()
    d_vones = nc.dram_tensor("vones", [LTP, HA], BF16, kind="ExternalInput").ap()
    d_sel1 = nc.dram_tensor("sel1", [HC, 4, 128], BF16, kind="ExternalInput").ap()
    d_sel3 = nc.dram_tensor("sel3", [HA, KD, 128], BF16, kind="ExternalInput").ap()
    d_out = nc.dram_tensor("out", [NV, D], F32, kind="ExternalOutput").ap()

    r128 = lambda ap: ap.rearrange("(kc p) n -> p kc n", p=128)
    LN8 = math.log(CAT_S)

    with tile.TileContext(nc) as tc, \
            nc.allow_low_precision(reason="fp8/bf16 rounding is intentional"):
        cst = tc.alloc_tile_pool(name="cst", bufs=1)
        ones_f = cst.tile([128, 128], F32, tag="ones_f")
        nc.vector.memset(ones_f[:], 1.0)
        ones_r = cst.tile([128, 1], F32R, tag="ones_r")
        nc.vector.tensor_copy(ones_r[:], ones_f[:, 0:1])
        ones_row_f = cst.tile([1, NP], F32, tag="ones_row_f")
        nc.vector.memset(ones_row_f[:], 1.0)
        neg_row_f = cst.tile([1, NP], F32, tag="neg_row_f")
        nc.vector.memset(neg_row_f[:], -1.0)
        ones_row = cst.tile([1, NP], F32R, tag="ones_row")
        nc.vector.tensor_copy(ones_row[:], ones_row_f[:])
        sel1 = cst.tile([HC, 4, 128], BF16, tag="sel1")
        nc.sync.dma_start(out=sel1[:], in_=d_sel1)
        sel3 = cst.tile([HA, KD, 128], BF16, tag="sel3")
        nc.sync.dma_start(out=sel3[:], in_=d_sel3)
        m2 = cst.tile([2, NP], F32R, tag="m2")
        nc.sync.dma_start(out=m2[1:2, :], in_=neg_row_f[:].bitcast(F32R))
        zeros_f = cst.tile([128, KD, 2], F32, tag="zeros_f")
        nc.vector.memset(zeros_f[:], 0.0)
        eps_t = cst.tile([1, 1], F32, tag="eps_t")
        nc.vector.memset(eps_t[:], EPS)
        ln8_t = cst.tile([HC, 1], F32, tag="ln8_t")
        nc.vector.memset(ln8_t[:], LN8)
        normgb = cst.tile([2, 3, KD, 128], F32R, tag="normgb")
        nc.sync.dma_start(out=normgb[:], in_=d_normgb)
        blin_t = cst.tile([128, KD], F32, tag="blin")
        nc.sync.dma_start(out=blin_t[:], in_=d_blin.rearrange("(kc p) -> p kc", p=128))
        bco_rows = cst.tile([1, KD, 128], F32R, tag="bco_rows")
        nc.sync.dma_start(out=bco_rows[:], in_=d_bco)
        bga_t = cst.tile([128, KF], F32, tag="bga")
        nc.sync.dma_start(out=bga_t[:], in_=d_bga.rearrange("(kc p) -> p kc", p=128))
        bgg_t = cst.tile([128, KF], F32, tag="bgg")
        nc.sync.dma_start(out=bgg_t[:], in_=d_bgg.rearrange("(kc p) -> p kc", p=128))
        bf_rows = cst.tile([1, KD, 128], F32R, tag="bf_rows")
        nc.sync.dma_start(out=bf_rows[:], in_=d_bf)
        boa_b = cst.tile([128, D], F32, tag="boa_b")
        nc.sync.dma_start(out=boa_b[:], in_=bass.AP(
            tensor=d_boa.tensor, offset=d_boa.offset, ap=[[0, 128]] + d_boa.ap))

        res = tc.alloc_tile_pool(name="res", bufs=1)  # hsT: lives phases 1-3
        hsT = res.tile([128, KD, NV], F32R, tag="hsT")

        def ln(x, out, n_tok, lidx):
            _emit_layernorm(tc, nc, x, out, n_tok, KD, lidx, normgb, m2, ones_r, eps_t)

        # ================= Phase 1: concat + LN1 + masked self-attention ======
        px = tc.alloc_tile_pool(name="px", bufs=1)
        xT = px.tile([128, KD, NP], F32R, tag="xT")
        for kc in range(KD):
            nc.sync.dma_start(out=xT[:, kc, 0:NV], in_=r128(d_hidT)[:, kc, :])
        nc.vector.tensor_copy(xT[:, :, N:NP], zeros_f[:])
        obj_sb = px.tile([128, KT, NOBJ], BF16, tag="obj_sb")
        nc.sync.dma_start(out=obj_sb[:], in_=r128(d_objT))
        with (
            tc.tile_pool(name="pwlin", bufs=1) as pwlin,
            tc.tile_pool(name="pps0", bufs=2, space="PSUM") as pps0,
        ):
            wlin = pwlin.tile([128, KT, D], BF16, tag="wlin")
            nc.sync.dma_start(out=wlin[:], in_=r128(d_wlin))
            for mc in range(KD):
                ps = pps0.tile([128, NOBJ], F32, tag="ps_obj")
                for kc in range(KT):
                    nc.tensor.matmul(ps[:], wlin[:, kc, mc * 128:(mc + 1) * 128],
                                     obj_sb[:, kc, :], start=(kc == 0), stop=(kc == KT - 1))
                nc.scalar.activation(xT[:, mc, NV:N], ps[:], AF.Identity,
                                     bias=blin_t[:, mc:mc + 1])

        pln1 = tc.alloc_tile_pool(name="pln1", bufs=1, side="right")
        ln1T = pln1.tile([128, KD, NP], FP8, tag="ln1T")
        ln(xT, ln1T, NP, 0)

        pqk = tc.alloc_tile_pool(name="pqk", bufs=1)
        pv1 = tc.alloc_tile_pool(name="pv1", bufs=1)
        qT = pqk.tile([128, 4, NP], BF16, tag="qT")
        kT = pqk.tile([128, 4, NP], BF16, tag="kT")
        v1 = pv1.tile([128, NJC, HC, CC + 1], BF16, tag="v1")
        nc.vector.memset(v1[:, :, :, CC:CC + 1], 1.0)
        with (
            tc.tile_pool(name="pwcma", bufs=2) as pwcma,
            tc.tile_pool(name="pps1", bufs=3, space="PSUM") as pps1,
        ):
            for d_w, dest, use_act in ((d_wq, qT, False), (d_wk, kT, True)):
                for half in range(2):
                    w8 = pwcma.tile([128, KD, 256], FP8, tag="w_cma")
                    nc.sync.dma_start(out=w8[:],
                                      in_=r128(d_w[:, half * 256:(half + 1) * 256]))
                    for mh in range(2):
                        mc = half * 2 + mh
                        for (io, iw) in IC_NP:
                            ps = pps1.tile([128, 512], F32, tag="ps_qk")
                            for kp in range(KD // 2):
                                nc.tensor.matmul(
                                    ps[:, :iw], w8[:, 2 * kp:2 * kp + 2, mh * 128:(mh + 1) * 128],
                                    ln1T[:, 2 * kp:2 * kp + 2, io:io + iw],
                                    start=(kp == 0), stop=(kp == KD // 2 - 1),
                                    perf_mode=PM.DoubleRow)
                            if use_act:
                                nc.scalar.activation(dest[:, mc, io:io + iw],
                                                     ps[:, :iw], AF.Copy,
                                                     scale=1.0 / SW_QKV)
                            else:
                                nc.vector.tensor_scalar_mul(dest[:, mc, io:io + iw],
                                                            ps[:, :iw], 1.0 / SW_QKV)
            for half in range(2):
                w8 = pwcma.tile([128, KD, 256], FP8, tag="w_cma")
                nc.sync.dma_start(out=w8[:], in_=r128(d_wv[:, half * 256:(half + 1) * 256]))
                for jc in range(NJC):
                    jw = 128 if jc < NJC - 1 else NP - 128 * (NJC - 1)
                    ps = pps1.tile([128, 256], F32, tag="ps_v")
                    for kp in range(KD // 2):
                        nc.tensor.matmul(ps[:jw, :],
                                         ln1T[:, 2 * kp:2 * kp + 2, jc * 128:jc * 128 + jw],
                                         w8[:, 2 * kp:2 * kp + 2, :],
                                         start=(kp == 0), stop=(kp == KD // 2 - 1),
                                         perf_mode=PM.DoubleRow)
                    nc.vector.tensor_scalar_mul(
                        v1[:jw, jc, half * 4:(half + 1) * 4, 0:CC],
                        ps[:jw, :].rearrange("p (h c) -> p h c", c=CC), 1.0 / SW_QKV)
        pln1.release()

        # Attention: simT[j,i] per head over the NV visual query columns only.
        pcat = tc.alloc_tile_pool(name="pcat", bufs=1, side="right")
        catR = pcat.tile([128, 4, NV], BF16, tag="catR")   # un-normalized
        catT = pcat.tile([128, 4, NV], FP8, tag="catT")    # catR * 8/den
        den8 = pcat.tile([HC, NV], F32, tag="den8")
        rden8 = pcat.tile([HC, NV], BF16, tag="rden8")
        with (
            tc.tile_pool(name="pm16", bufs=3) as pm16,
            tc.tile_pool(name="ppt", bufs=3) as ppt,
            tc.tile_pool(name="pden", bufs=1) as pden,
        ):
            with (
                tc.tile_pool(name="psim", bufs=2, space="PSUM") as psim,
                tc.tile_pool(name="pav", bufs=2, space="PSUM") as pav,
            ):
                for h in range(HC):
                    pr = (h % 2) * 64
                    hc = h // 2
                    av_ps = pav.tile([CC + 1, NV], F32, tag="ps_av")
                    for jc in range(NJC):
                        jw = 128 if jc < NJC - 1 else NP - 128 * (NJC - 1)
                        m16 = pm16.tile([128, NV], BF16, tag="m16")
                        nc.sync.dma_start(out=m16[:jw, :],
                                          in_=d_mask[h, jc * 128:jc * 128 + jw, :])
                        ps_s = psim.tile([128, NV], F32, tag="ps_sim")
                        for (io, iw) in IC_NV:
                            nc.tensor.matmul(ps_s[:jw, io:io + iw],
                                             kT[pr:pr + 64, hc, jc * 128:jc * 128 + jw],
                                             qT[pr:pr + 64, hc, io:io + iw],
                                             start=True, stop=True)
                        pt = ppt.tile([128, NV], BF16, tag="pt")
                        nc.scalar.activation(pt[:jw, :], ps_s[:jw, :], AF.Exp,
                                             scale=SCALE)
                        ptm = ppt.tile([128, NV], BF16, tag="ptm")
                        nc.vector.tensor_mul(ptm[:jw, :], pt[:jw, :], m16[:jw, :])
                        for (io, iw) in IC_NV:
                            nc.tensor.matmul(av_ps[:, io:io + iw], v1[:jw, jc, h, :],
                                             ptm[:jw, io:io + iw],
                                             start=(jc == 0), stop=(jc == NJC - 1))
                    nc.vector.tensor_copy(catR[pr:pr + 64, hc, :], av_ps[0:CC, :])
                    den_st = pden.tile([1, NV], F32, tag="den_st")
                    nc.vector.tensor_copy(den_st[:], av_ps[CC:CC + 1, :])
                    nc.sync.dma_start(out=den8[h:h + 1, :], in_=den_st[:])
            # batched reciprocal: rden = 8/den via exp(-ln(den)+ln8) on ACT
            dln = pden.tile([HC, NV], F32, tag="dln")
            nc.scalar.activation(dln[:], den8[:], AF.Ln)
            nc.scalar.activation(rden8[:], dln[:], AF.Exp, scale=-1.0, bias=ln8_t[:])
            with tc.tile_pool(name="pdbc", bufs=2, space="PSUM") as pdbc:
                for hc in range(4):
                    for (io, iw) in IC_NV:
                        pd = pdbc.tile([128, iw], F32, tag="pd")
                        nc.tensor.matmul(pd[:], sel1[:, hc, :], rden8[:, io:io + iw],
                                         start=True, stop=True)
                        nc.vector.tensor_mul(catT[:, hc, io:io + iw],
                                             catR[:, hc, io:io + iw], pd[:])
        pv1.release()
        pqk.release()

        # Output projection (tanh/2048-folded) + residual into hsT.
        with (
            tc.tile_pool(name="pwco", bufs=1) as pwco,
            tc.tile_pool(name="pco", bufs=3, space="PSUM") as pco,
        ):
            w_co8 = pwco.tile([128, 4, D], FP8, tag="w_co8")
            nc.sync.dma_start(out=w_co8[:], in_=r128(d_wco))
            for mc in range(KD):
                for (io, iw) in IC_NV:
                    ps = pco.tile([128, iw], F32, tag="ps_co")
                    for kp in range(2):
                        nc.tensor.matmul(ps[:], w_co8[:, 2 * kp:2 * kp + 2, mc * 128:(mc + 1) * 128],
                                         catT[:, 2 * kp:2 * kp + 2, io:io + iw],
                                         start=(kp == 0), stop=False,
                                         perf_mode=PM.DoubleRow)
                    # bias via rank-1 matmul (bco prescaled by SW_CO*CAT_S host-side)
                    nc.tensor.matmul(ps[:], bco_rows[:, mc, :], ones_row[:, io:io + iw],
                                     start=False, stop=True)
                    nc.vector.scalar_tensor_tensor(
                        out=hsT[:, mc, io:io + iw], in0=ps[:],
                        scalar=1.0 / (SW_CO * CAT_S),
                        in1=xT[:, mc, io:io + iw].bitcast(F32),
                        op0=ALU.mult, op1=ALU.add)
        pcat.release()
        px.release()  # xT dead

        # ================= Phase 2: LN2 + GEGLU FFN (fp8) =====================
        pln2 = tc.alloc_tile_pool(name="pln2", bufs=1)
        ln2T = pln2.tile([128, KD, NV], FP8, tag="ln2T")
        ln(hsT, ln2T, NV, 1)
        pff = tc.alloc_tile_pool(name="pff", bufs=1, side="right")
        ffT = pff.tile([128, KF, NV], FP8, tag="ffT")    # 16*(a+b)*gelu
        with (
            tc.tile_pool(name="pwg", bufs=2) as pwg,
            tc.tile_pool(name="p2s", bufs=3) as p2s,
            tc.tile_pool(name="p2ps", bufs=2, space="PSUM") as p2ps,
        ):
            for m in range(KF):
                if m % 2 == 0:
                    wga = pwg.tile([128, KD, 256], FP8, tag="wga")
                    nc.sync.dma_start(out=wga[:], in_=r128(d_wg[:, m * 128:(m + 2) * 128]))
                    wgg = pwg.tile([128, KD, 256], FP8, tag="wgg")
                    nc.sync.dma_start(out=wgg[:],
                                      in_=r128(d_wg[:, DFF + m * 128:DFF + (m + 2) * 128]))
                mo = (m % 2) * 128
                ps_a = p2ps.tile([128, NV], F32, tag="ps_a")
                ps_g = p2ps.tile([128, NV], F32, tag="ps_g")
                for kp in range(KD // 2):
                    for (io, iw) in IC_NV:
                        nc.tensor.matmul(ps_a[:, io:io + iw], wga[:, 2 * kp:2 * kp + 2, mo:mo + 128],
                                         ln2T[:, 2 * kp:2 * kp + 2, io:io + iw],
                                         start=(kp == 0), stop=(kp == KD // 2 - 1),
                                         perf_mode=PM.DoubleRow)
                    for (io, iw) in IC_NV:
                        nc.tensor.matmul(ps_g[:, io:io + iw], wgg[:, 2 * kp:2 * kp + 2, mo:mo + 128],
                                         ln2T[:, 2 * kp:2 * kp + 2, io:io + iw],
                                         start=(kp == 0), stop=(kp == KD // 2 - 1),
                                         perf_mode=PM.DoubleRow)
                gelu_sb = p2s.tile([128, NV], BF16, tag="gelu_sb")
                nc.scalar.activation(gelu_sb[:], ps_g[:], GELU_AF,
                                     scale=1.0 / SW_G, bias=bgg_t[:, m:m + 1])
                nc.vector.scalar_tensor_tensor(
                    out=ffT[:, m, :], in0=ps_a[:], scalar=bga_t[:, m:m + 1],
                    in1=gelu_sb[:], op0=ALU.add, op1=ALU.mult)
        pln2.release()
        # ffout (tanh/2048-folded) + residual in place.
        with (
            tc.tile_pool(name="pwf", bufs=2) as pwf,
            tc.tile_pool(name="pfps", bufs=3, space="PSUM") as pfps,
        ):
            for mc in range(KD):
                if mc % 2 == 0:
                    wf = pwf.tile([128, KF, 256], FP8, tag="wf")
                    nc.sync.dma_start(out=wf[:], in_=r128(d_wf[:, mc * 128:(mc + 2) * 128]))
                mo = (mc % 2) * 128
                for (io, iw) in IC_NV:
                    ps = pfps.tile([128, iw], F32, tag="ps_f")
                    for kp in range(KF // 2):
                        nc.tensor.matmul(ps[:], wf[:, 2 * kp:2 * kp + 2, mo:mo + 128],
                                         ffT[:, 2 * kp:2 * kp + 2, io:io + iw],
                                         start=(kp == 0), stop=False,
                                         perf_mode=PM.DoubleRow)
                    # bias via rank-1 matmul (bf prescaled by SW_F*SW_G host-side)
                    nc.tensor.matmul(ps[:], bf_rows[:, mc, :], ones_row[:, io:io + iw],
                                     start=False, stop=True)
                    nc.vector.scalar_tensor_tensor(
                        out=hsT[:, mc, io:io + iw], in0=ps[:],
                        scalar=1.0 / (SW_F * SW_G), in1=hsT[:, mc, io:io + iw].bitcast(F32),
                        op0=ALU.mult, op1=ALU.add)
        pff.release()

        # ================= Phase 3: LN3 + cross-attention (baseline impl) =====
        pln3 = tc.alloc_tile_pool(name="pln3", bufs=1, side="right")
        ln3T = pln3.tile([128, KD, NV], BF16, tag="ln3T")
        ln(hsT, ln3T, NV, 2)
        res.release()  # hsT dead

        pp3 = tc.alloc_tile_pool(name="pp3", bufs=2, space="PSUM")
        pq3 = tc.alloc_tile_pool(name="pq3", bufs=1)
        qTa = pq3.tile([128, KD, NV], BF16, tag="qTa")
        with tc.tile_pool(name="pwqa", bufs=2) as pwqa:
            for mc in range(KD):
                if mc % 2 == 0:
                    wqa = pwqa.tile([128, KD, 256], BF16, tag="wqa")
                    nc.sync.dma_start(out=wqa[:],
                                      in_=r128(d_wqa[:, mc * 128:(mc + 2) * 128]))
                mo = (mc % 2) * 128
                for (io, iw) in IC_NV:
                    ps = pp3.tile([128, iw], F32, tag="ps_p3")
                    for kc in range(KD):
                        nc.tensor.matmul(ps[:], wqa[:, kc, mo:mo + 128],
                                         ln3T[:, kc, io:io + iw],
                                         start=(kc == 0), stop=(kc == KD - 1))
                    nc.vector.tensor_copy(qTa[:, mc, io:io + iw], ps[:])
        pln3.release()

        penc = tc.alloc_tile_pool(name="penc", bufs=1)
        enc_sb = penc.tile([128, KT, LTP], F32R, tag="enc_sb")
        nc.sync.dma_start(out=enc_sb[:], in_=r128(d_encT))
        kTa = penc.tile([128, KD, LTP], BF16, tag="kTa")
        v1a = penc.tile([LTP, HA, CA + 1], BF16, tag="v1a")
        nc.sync.dma_start(out=v1a[:, :, CA:CA + 1], in_=d_vones.unsqueeze(2))
        with tc.tile_pool(name="pwenc", bufs=1) as pwenc:
            wka = pwenc.tile([128, KT, INNER_A], F32R, tag="w_enc")
            nc.sync.dma_start(out=wka[:], in_=r128(d_wka))
            for mc in range(KD):
                ps = pp3.tile([128, LTP], F32, tag="ps_p3")
                for kc in range(KT):
                    nc.tensor.matmul(ps[:], wka[:, kc, mc * 128:(mc + 1) * 128],
                                     enc_sb[:, kc, :], start=(kc == 0), stop=(kc == KT - 1))
                nc.vector.tensor_copy(kTa[:, mc, :], ps[:])
            wva = pwenc.tile([128, KT, INNER_A], F32R, tag="w_enc")
            nc.sync.dma_start(out=wva[:], in_=r128(d_wva))
            for (co, cw) in DC_D:
                ps = pp3.tile([LTP, cw], F32, tag="ps_p3")
                for kc in range(KT):
                    nc.tensor.matmul(ps[:], enc_sb[:, kc, :], wva[:, kc, co:co + cw],
                                     start=(kc == 0), stop=(kc == KT - 1))
                nc.vector.tensor_copy(v1a[:, co // CA:(co + cw) // CA, 0:CA],
                                      ps[:].rearrange("p (h c) -> p h c", c=CA))

        pcat3 = tc.alloc_tile_pool(name="pcat3", bufs=1, side="right")
        catTa = pcat3.tile([128, KD, NV], F32R, tag="catTa")
        catB = pcat3.tile([128, KD, NV], BF16, tag="catB")
        den20 = pcat3.tile([HA, NV], F32, tag="den20")
        rden20 = pcat3.tile([HA, NV], BF16, tag="rden20")
        pp3.release()
        with (
            tc.tile_pool(name="p3s", bufs=3) as p3s,
            tc.tile_pool(name="pden3", bufs=1) as pden3,
        ):
            with (
                tc.tile_pool(name="psa", bufs=2, space="PSUM") as psa,
                tc.tile_pool(name="pava", bufs=3, space="PSUM") as pava,
            ):
                ptas = {}

                def emit_sim3(h):
                    pr = (h % 2) * 64
                    hc = h // 2
                    pta = p3s.tile([LTP, NV], BF16, tag="pta")
                    for (io, iw) in IC_NV:
                        ps_s = psa.tile([LTP, iw], F32, tag="ps_sa")
                        nc.tensor.matmul(ps_s[:], kTa[pr:pr + 64, hc, :],
                                         qTa[pr:pr + 64, hc, io:io + iw],
                                         start=True, stop=True)
                        nc.scalar.activation(pta[:, io:io + iw], ps_s[:], AF.Exp,
                                             scale=SCALE)
                    ptas[h] = pta

                def emit_av3(h):
                    pr = (h % 2) * 64
                    hc = h // 2
                    pta = ptas.pop(h)
                    for (io, iw) in IC_NV:
                        ps_av = pava.tile([CA + 1, iw], F32, tag="ps_ava")
                        nc.tensor.matmul(ps_av[:], v1a[:, h, :], pta[:, io:io + iw],
                                         start=True, stop=True)
                        if h % 2 == 0:
                            nc.vector.tensor_copy(catTa[pr:pr + 64, hc, io:io + iw],
                                                  ps_av[0:CA, :])
                        else:
                            nc.scalar.activation(catTa[pr:pr + 64, hc, io:io + iw],
                                                 ps_av[0:CA, :], AF.Copy)
                        den_st = pden3.tile([1, 512], F32, tag="den_st3")
                        nc.vector.tensor_copy(den_st[:, :iw], ps_av[CA:CA + 1, :])
                        nc.sync.dma_start(out=den20[h:h + 1, io:io + iw],
                                          in_=den_st[:, :iw])

                emit_sim3(0)
                for h in range(1, HA):
                    emit_sim3(h)
                    emit_av3(h - 1)
                emit_av3(HA - 1)
            dln = pden3.tile([HA, NV], F32, tag="dln20")
            nc.scalar.activation(dln[:], den20[:], AF.Ln)
            nc.scalar.activation(rden20[:], dln[:], AF.Exp, scale=-1.0)
            with tc.tile_pool(name="pdbca", bufs=2, space="PSUM") as pdbca:
                for hc in range(KD):
                    for (io, iw) in IC_NV:
                        pd = pdbca.tile([128, iw], F32, tag="pda")
                        nc.tensor.matmul(pd[:], sel3[:, hc, :],
                                         rden20[:, io:io + iw],
                                         start=True, stop=True)
                        nc.vector.tensor_mul(catB[:, hc, io:io + iw],
                                             catTa[:, hc, io:io + iw].bitcast(F32),
                                             pd[:])
        penc.release()
        pq3.release()

        # Final projection, token-major out (stationary = catB chunks).
        with (
            tc.tile_pool(name="pwoa", bufs=1) as pwoa,
            tc.tile_pool(name="po", bufs=3) as po,
            tc.tile_pool(name="poo", bufs=2, space="PSUM") as poo,
        ):
            woa = pwoa.tile([128, KD, D], BF16, tag="woa")
            nc.sync.dma_start(out=woa[:], in_=r128(d_woa))
            for it in range(NV // 128):
                pss = [poo.tile([128, dcw], F32, tag=f"ps_oo{i}", name=f"ps_oo{i}")
                       for i, (dco, dcw) in enumerate(DC_D)]
                for kc in range(KD):
                    for i, (dco, dcw) in enumerate(DC_D):
                        nc.tensor.matmul(pss[i][:], catB[:, kc, it * 128:(it + 1) * 128],
                                         woa[:, kc, dco:dco + dcw],
                                         start=(kc == 0), stop=(kc == KD - 1))
                for i, (dco, dcw) in enumerate(DC_D):
                    o_sb = po.tile([128, dcw], F32, tag="o_sb")
                    nc.vector.tensor_add(o_sb[:], pss[i][:], boa_b[:, dco:dco + dcw])
                    nc.sync.dma_start(out=d_out[it * 128:(it + 1) * 128, dco:dco + dcw],
                                      in_=o_sb[:])
        pcat3.release()
        cst.release()

    nc.compile()
    return nc


_CACHE = {}


def _get_nc():
    if "nc" not in _CACHE:
        _CACHE["nc"] = build_nc()
    return _CACHE["nc"]


def _sel(nh):
    bf16 = ml_dtypes.bfloat16
    s = np.zeros((nh, nh // 2, 128), dtype=bf16)
    for hc in range(nh // 2):
        s[2 * hc, hc, 0:64] = 1
        s[2 * hc + 1, hc, 64:128] = 1
    return s


def prepare_in_maps(inputs):
    f32 = np.float32
    bf16 = ml_dtypes.bfloat16
    fp8 = ml_dtypes.float8_e4m3
    hidT = np.ascontiguousarray(inputs["hidden_states"].transpose(0, 2, 1), dtype=f32)
    objT = np.ascontiguousarray(inputs["object_embeddings"].transpose(0, 2, 1)).astype(bf16)
    encT = np.zeros((B, DTXT, LTP), dtype=f32)
    encT[:, :, :LTXT] = inputs["encoder_hidden_states"].transpose(0, 2, 1)
    masks = inputs["object_attention_masks"]
    maskTb = np.zeros((B, HC, NP, NV), dtype=bf16)
    maskTb[:, :, :N, :] = (masks.transpose(0, 1, 3, 2)[:, :, :, :NV] > 0)

    ta = float(np.tanh(inputs["alpha_attn"]))
    td = float(np.tanh(inputs["alpha_dense"]))
    w_co8 = (np.asarray(inputs["cma_out_w"]) * (ta * SW_CO)).astype(fp8)
    bco_rows = (np.asarray(inputs["cma_out_b"]) * (ta * SW_CO * CAT_S)).astype(
        f32).reshape(1, KD, 128)
    w_f8 = (np.asarray(inputs["ffout_w"]) * (td * SW_F)).astype(fp8)
    bf_rows = (np.asarray(inputs["ffout_b"]) * (td * SW_F * SW_G)).astype(
        f32).reshape(1, KD, 128)
    bg = np.asarray(inputs["geglu_b"], dtype=f32)
    norms = [inputs["norm1_g"], inputs["norm1_b"], inputs["norm2_g"],
             inputs["norm2_b"], inputs["norm3_g"], inputs["norm3_b"]]
    normgb = np.stack([np.stack(norms[0::2]), np.stack(norms[1::2])]).astype(f32)
    shared = {
        "w_lin": np.asarray(inputs["linear_w"]).astype(bf16),
        "b_lin": np.ascontiguousarray(inputs["linear_b"], dtype=f32),
        "w_q8": (np.asarray(inputs["cma_q_w"]) * SW_QKV).astype(fp8),
        "w_k8": (np.asarray(inputs["cma_k_w"]) * SW_QKV).astype(fp8),
        "w_v8": (np.asarray(inputs["cma_v_w"]) * SW_QKV).astype(fp8),
        "w_co8": w_co8, "bco_rows": bco_rows,
        "w_g8": (np.asarray(inputs["geglu_w"]) * SW_G).astype(fp8),
        "bg_a16": (bg[:DFF] * SW_G),
        "bg_g": bg[DFF:],
        "w_f8": w_f8, "bf_rows": bf_rows,
        "w_qa8": np.asarray(inputs["attn_q_w"]).astype(bf16),
        "w_ka": np.ascontiguousarray(inputs["attn_k_w"], dtype=f32),
        "w_va": np.ascontiguousarray(inputs["attn_v_w"], dtype=f32),
        "w_oa": np.asarray(inputs["attn_out_w"]).astype(bf16),
        "b_oa": np.ascontiguousarray(inputs["attn_out_b"], dtype=f32),
        "normgb": normgb.reshape(2, 3, KD, 128),
        "vones": np.concatenate([np.ones((LTXT, HA)),
                                 np.zeros((LTP - LTXT, HA))], axis=0).astype(bf16),
        "sel1": _sel(HC), "sel3": _sel(HA),
    }
    in_maps = []
    for b in range(B):
        m = dict(shared)
        m["hidT"] = hidT[b]
        m["objT"] = objT[b]
        m["encT"] = encT[b]
        m["maskTb"] = np.ascontiguousarray(maskTb[b])
        in_maps.append(m)
    return in_maps


def run(inputs, trace=False):
    nc = _get_nc()
    in_maps = prepare_in_maps(inputs)
    res = run_bass_kernel_spmd(nc, in_maps, core_ids=list(range(B)), trace=trace)
    out = np.stack([res.results[b]["out"] for b in range(B)], axis=0)
    return out, res


def kernel(**inputs):
    out, _ = run(inputs, trace=False)
    return out



# revision 21
# speedup vs baseline: 1.1254x; 1.1254x over previous
"""Trainium2 Bass kernel for nn_CustomAttnProcessor (dense transformer block).

Data-parallel over batch B=8 across 8 NeuronCores; one batch element per core.

v2 layout (channel-major activations: [feature_partition, token_free]):
  - LN gammas folded into downstream weights host-side; betas folded into
    downstream biases (k-side betas drop out of softmax). On-device LN only
    computes x_hat = (x - mu) * rstd via 2 broadcast matmuls per token chunk.
  - LN stats (mu, E[x^2]) col-packed at PSUM partitions 0/32 (2x concurrency).
  - QKV / FFN2 loops are kp-outer so LDWEIGHTS amortizes over token chunks.
  - attn1 heads processed in even/odd pairs at PE row groups 0/64.
  - attn3: per-head fat exp, denominators normalized in 2 head groups so the
    final projection's contraction runs as 2 passes (kc 0-4 / 5-9), the first
    overlapping the second half of attention.
  - biases applied via ACT Identity into PSUM / DVE tensor_scalar, not rank-1
    matmuls.
"""

import math
import os
import sys

import numpy as np
import ml_dtypes

sys.path.insert(0, "/opt/trn_rl_repo")

import concourse.bass as bass
import concourse.tile as tile
from concourse import bacc, mybir
from concourse.bass_utils import run_bass_kernel_spmd

F32 = mybir.dt.float32
F32R = mybir.dt.float32r
BF16 = mybir.dt.bfloat16
FP8 = mybir.dt.float8e4
AF = mybir.ActivationFunctionType
ALU = mybir.AluOpType
PM = mybir.MatmulPerfMode
GELU_AF = AF.Tanh if os.environ.get("SIM_SAFE_GELU") else AF.Gelu

B = 8
NV = 1024          # visual tokens (the only query columns computed)
NOBJ = 30
N = NV + NOBJ      # 1054
NP = 1056          # padded token count (keys)
NJC = 9            # key-dim 128-chunks over NP (last chunk = 32 rows)
D = 1280
KD = D // 128      # 10
DTXT = 768
KT = DTXT // 128   # 6
LTXT = 77
LTP = 78           # padded
HC, CC = 8, 64     # masked self-attention heads
HA, CA = 20, 64    # cross-attention heads
INNER_C = HC * CC  # 512
INNER_A = HA * CA  # 1280
DFF = 4 * D        # 5120
KF = DFF // 128    # 40
EPS = 1e-5
SCALE = CC ** -0.5  # 0.125

SW_QKV = 256.0     # w_q/k/v, w_qa
SW_CO = 2048.0     # w_co (tanh-folded)
SW_G = 16.0        # w_geglu (also leaves ffT scaled x16 for fp8 range)
SW_F = 2048.0      # w_ffout (tanh-folded)
CAT_S = 8.0        # catT = 8*attn (keeps fp8 out of denormals)

IC_NP = [(0, 512), (512, 512), (1024, 32)]   # token chunks for 1056
IC_NV = [(0, 512), (512, 512)]               # token chunks for 1024
DC_D = [(0, 512), (512, 512), (1024, 256)]   # feature chunks for 1280


def _emit_layernorm2(tc, nc, xs, outs, n_tok, ones_r, ones_row128, eps_t,
                     st_name):
    """x_hat = (x - mu) * rstd over the partition (feature) axis.

    xs:   list of KD tiles [128, >=n_tok] (only [:, :n_tok] read)
    outs: list of per-chunk tiles [128, KD, w] (fp8/bf16)
    """
    chunks = [(o, w) for (o, w) in IC_NP if o < n_tok]
    nch = len(chunks)
    inv_d = 1.0 / float(KD * 128)
    with (
        tc.tile_pool(name=f"{st_name}_rows", bufs=1) as rows,
        tc.tile_pool(name=f"{st_name}_t1", bufs=3) as t1p,
    ):
        mu3 = rows.tile([1, nch, 512], F32, tag="mu3")
        ex3 = rows.tile([1, nch, 512], F32, tag="ex3")
        var3 = rows.tile([1, nch, 512], F32, tag="var3")
        rs3 = rows.tile([1, nch, 512], F32R, tag="rs3")
        murs3 = rows.tile([1, nch, 512], F32R, tag="murs3")
        with (
            tc.tile_pool(name=f"{st_name}_st", bufs=1, space="PSUM") as stp,
            tc.tile_pool(name=f"{st_name}_sq", bufs=2) as sqp,
        ):
            st_mu = stp.tile([1, nch, 512], F32, tag="st_mu")
            st_ex = stp.tile([1, nch, 512], F32, tag="st_ex")
            for kc in range(KD):
                sq = sqp.tile([128, n_tok], BF16, tag="ln_sq")
                nc.scalar.activation(sq[:], xs[kc][:, 0:n_tok], AF.Square)
                for i, (o, w) in enumerate(chunks):
                    nc.tensor.matmul(st_mu[:, i, :w], ones_r[:],
                                     xs[kc][:, o:o + w],
                                     start=(kc == 0), stop=(kc == KD - 1))
                    nc.tensor.matmul(st_ex[:, i, :w], ones_r[:], sq[:, o:o + w],
                                     start=(kc == 0), stop=(kc == KD - 1))
            nc.vector.tensor_scalar_mul(mu3[:], st_mu[:], inv_d)
            nc.vector.tensor_scalar_mul(ex3[:], st_ex[:], inv_d)
        nc.vector.tensor_mul(var3[:], mu3[:], mu3[:])
        nc.vector.tensor_sub(var3[:], ex3[:], var3[:])
        # rstd = exp(-0.5*ln(var+eps))
        nc.scalar.activation(ex3[:], var3[:], AF.Ln, bias=eps_t[:])
        nc.scalar.activation(rs3[:], ex3[:], AF.Exp, scale=-0.5)
        nc.vector.tensor_mul(murs3[:], mu3[:], rs3[:].bitcast(F32))
        with (
            tc.tile_pool(name=f"{st_name}_bc", bufs=2, space="PSUM") as bcp,
            tc.tile_pool(name=f"{st_name}_bcs", bufs=2) as bcsp,
        ):
            for i, (o, w) in enumerate(chunks):
                bcr = bcp.tile([128, 512], F32, tag="bcr")
                bcm = bcp.tile([128, 512], F32, tag="bcm")
                nc.tensor.matmul(bcr[:, :w], ones_row128[:], rs3[:, i, :w],
                                 start=True, stop=True)
                nc.tensor.matmul(bcm[:, :w], ones_row128[:],
                                 murs3[:, i, :w], start=True, stop=True)
                # stage broadcasts to SBUF bf16 so the apply runs at DVE 2x
                bcr_s = bcsp.tile([128, 512], BF16, tag="bcr_s")
                bcm_s = bcsp.tile([128, 512], BF16, tag="bcm_s")
                nc.scalar.copy(bcr_s[:, :w], bcr[:, :w])
                nc.scalar.copy(bcm_s[:, :w], bcm[:, :w])
                for kc in range(KD):
                    eng = nc.vector if kc < 7 else nc.gpsimd
                    t1 = t1p.tile([128, 512], BF16, tag="ln_t1")
                    eng.tensor_mul(t1[:, :w], xs[kc][:, o:o + w], bcr_s[:, :w])
                    eng.tensor_sub(outs[i][:, kc, :w], t1[:, :w], bcm_s[:, :w])


def build_nc():
    nc = bacc.Bacc("TRN2", target_bir_lowering=False, debug=False, num_devices=B)

    # ---- DRAM I/O (per core) ----
    d_hidT = nc.dram_tensor("hidT", [D, NV], BF16, kind="ExternalInput").ap()
    d_objT = nc.dram_tensor("objT", [DTXT, NOBJ], BF16, kind="ExternalInput").ap()
    d_encT = nc.dram_tensor("encT", [DTXT, LTP], BF16, kind="ExternalInput").ap()
    d_mask = nc.dram_tensor("mask8", [HC, NP, NV], BF16, kind="ExternalInput").ap()
    d_wlin = nc.dram_tensor("w_lin", [DTXT, D], BF16, kind="ExternalInput").ap()
    d_blin = nc.dram_tensor("b_lin", [D], F32, kind="ExternalInput").ap()
    d_wq = nc.dram_tensor("w_q8", [D, INNER_C], FP8, kind="ExternalInput").ap()
    d_wk = nc.dram_tensor("w_k8", [D, INNER_C], FP8, kind="ExternalInput").ap()
    d_wv = nc.dram_tensor("w_v8", [D, INNER_C], FP8, kind="ExternalInput").ap()
    d_qb = nc.dram_tensor("qb_cols", [128, 4], F32, kind="ExternalInput").ap()
    d_wco = nc.dram_tensor("w_co8", [INNER_C, D], FP8, kind="ExternalInput").ap()
    d_bco = nc.dram_tensor("bco_cols", [128, KD], F32, kind="ExternalInput").ap()
    d_wg = nc.dram_tensor("w_g8", [D, 2 * DFF], FP8, kind="ExternalInput").ap()
    d_bga = nc.dram_tensor("bg_a16", [DFF], F32, kind="ExternalInput").ap()
    d_bgg = nc.dram_tensor("bg_g", [DFF], F32, kind="ExternalInput").ap()
    d_wf = nc.dram_tensor("w_f8", [DFF, D], FP8, kind="ExternalInput").ap()
    d_bf = nc.dram_tensor("bf_cols", [128, KD], F32, kind="ExternalInput").ap()
    d_wqa = nc.dram_tensor("w_qa8", [D, INNER_A], BF16, kind="ExternalInput").ap()
    d_qab = nc.dram_tensor("qab_cols", [128, KD], F32, kind="ExternalInput").ap()
    d_wka = nc.dram_tensor("w_ka", [DTXT, INNER_A], BF16, kind="ExternalInput").ap()
    d_wva = nc.dram_tensor("w_va", [DTXT, INNER_A], BF16, kind="ExternalInput").ap()
    d_woa = nc.dram_tensor("w_oa", [INNER_A, D], BF16, kind="ExternalInput").ap()
    d_boa = nc.dram_tensor("b_oa", [D], F32, kind="ExternalInput").ap()
    d_vones = nc.dram_tensor("vones", [LTP, HA], BF16, kind="ExternalInput").ap()
    d_sel1 = nc.dram_tensor("sel1", [HC, 4, 128], BF16, kind="ExternalInput").ap()
    d_sel3 = nc.dram_tensor("sel3", [HA, KD, 128], BF16, kind="ExternalInput").ap()
    d_out = nc.dram_tensor("out", [NV, D], F32, kind="ExternalOutput").ap()

    r128 = lambda ap: ap.rearrange("(kc p) n -> p kc n", p=128)
    LN8 = math.log(CAT_S)

    with tile.TileContext(nc) as tc, \
            nc.allow_low_precision(reason="fp8/bf16 rounding is intentional"):
        cst = tc.alloc_tile_pool(name="cst", bufs=1)
        ones_f = cst.tile([128, 128], F32, tag="ones_f")
        nc.vector.memset(ones_f[:], 1.0)
        ones_r = cst.tile([128, 1], BF16, tag="ones_r")
        nc.vector.tensor_copy(ones_r[:], ones_f[:, 0:1])
        ones_row128 = cst.tile([1, 128], F32R, tag="ones_row128")
        nc.vector.tensor_copy(ones_row128[:], ones_f[0:1, :])
        zeros2 = cst.tile([128, 2], BF16, tag="zeros2")
        nc.vector.memset(zeros2[:], 0.0)
        eps_t = cst.tile([1, 1], F32, tag="eps_t")
        nc.vector.memset(eps_t[:], EPS)
        ln8_t = cst.tile([HC, 1], F32, tag="ln8_t")
        nc.vector.memset(ln8_t[:], LN8)
        sel1 = cst.tile([HC, 4, 128], BF16, tag="sel1")
        nc.sync.dma_start(out=sel1[:], in_=d_sel1)
        sel3 = cst.tile([HA, KD, 128], BF16, tag="sel3")
        nc.sync.dma_start(out=sel3[:], in_=d_sel3)
        blin_t = cst.tile([128, KD], F32, tag="blin")
        nc.sync.dma_start(out=blin_t[:], in_=d_blin.rearrange("(kc p) -> p kc", p=128))
        qb_t = cst.tile([128, 4], F32, tag="qb_t")
        nc.sync.dma_start(out=qb_t[:], in_=d_qb)
        bco_t = cst.tile([128, KD], F32, tag="bco_t")
        nc.sync.dma_start(out=bco_t[:], in_=d_bco)
        bga_t = cst.tile([128, KF], F32, tag="bga")
        nc.sync.dma_start(out=bga_t[:], in_=d_bga.rearrange("(kc p) -> p kc", p=128))
        bgg_t = cst.tile([128, KF], F32, tag="bgg")
        nc.sync.dma_start(out=bgg_t[:], in_=d_bgg.rearrange("(kc p) -> p kc", p=128))
        bf_t = cst.tile([128, KD], F32, tag="bf_t")
        nc.sync.dma_start(out=bf_t[:], in_=d_bf)
        qab_t = cst.tile([128, KD], F32, tag="qab_t")
        nc.sync.dma_start(out=qab_t[:], in_=d_qab)
        boa_b = cst.tile([128, D], F32, tag="boa_b")
        nc.sync.dma_start(out=boa_b[:], in_=bass.AP(
            tensor=d_boa.tensor, offset=d_boa.offset, ap=[[0, 128]] + d_boa.ap))

        res = tc.alloc_tile_pool(name="res", bufs=1)  # hsT per-mc: phases 1-3
        hsT = [res.tile([128, NV], F32R, tag=f"hsT{mc}") for mc in range(KD)]

        # ================= Phase 1: concat + LN1 + masked self-attention ======
        px = tc.alloc_tile_pool(name="px", bufs=1)
        obj_sb = px.tile([128, KT, NOBJ], BF16, tag="obj_sb")
        nc.sync.dma_start(out=obj_sb[:], in_=r128(d_objT))
        xT = [px.tile([128, NP], F32R, tag=f"xT{kc}") for kc in range(KD)]
        with (
            tc.tile_pool(name="pwlin", bufs=1) as pwlin,
            tc.tile_pool(name="pps0", bufs=2, space="PSUM") as pps0,
        ):
            wlin = pwlin.tile([128, KT, D], BF16, tag="wlin")
            for kc in range(KT):
                nc.sync.dma_start(out=wlin[:, kc, :], in_=r128(d_wlin)[:, kc, :])
            for mc in range(KD):
                nc.sync.dma_start(out=xT[mc][:, 0:NV], in_=r128(d_hidT)[:, mc, :])
                ps = pps0.tile([128, NOBJ], F32, tag="ps_obj")
                for kc in range(KT):
                    nc.tensor.matmul(ps[:], wlin[:, kc, mc * 128:(mc + 1) * 128],
                                     obj_sb[:, kc, :], start=(kc == 0),
                                     stop=(kc == KT - 1))
                nc.scalar.activation(xT[mc][:, NV:N], ps[:], AF.Identity,
                                     bias=blin_t[:, mc:mc + 1])
                nc.vector.tensor_copy(xT[mc][:, N:NP], zeros2[:])

        pln1 = tc.alloc_tile_pool(name="pln1", bufs=1, side="right")
        ln1c = [pln1.tile([128, KD, w], FP8, tag=f"ln1c{i}")
                for i, (o, w) in enumerate(IC_NP)]
        _emit_layernorm2(tc, nc, xT, ln1c, NP, ones_r, ones_row128, eps_t, "ln1")

        pqk = tc.alloc_tile_pool(name="pqk", bufs=1)
        pv1 = tc.alloc_tile_pool(name="pv1", bufs=1)
        qT = pqk.tile([128, 4, NP], BF16, tag="qT")
        kT = pqk.tile([128, 4, NP], BF16, tag="kT")
        v1 = pv1.tile([128, NJC, HC, CC + 1], BF16, tag="v1")
        nc.vector.memset(v1[:, :, :, CC:CC + 1], 1.0)
        with (
            tc.tile_pool(name="pwcv", bufs=2) as pwcv,
            tc.tile_pool(name="ppsv", bufs=2, space="PSUM") as ppsv,
        ):
            w8s = []
            for half in range(2):
                w8 = pwcv.tile([128, KD, 256], FP8, tag="w_cv")
                nc.sync.dma_start(out=w8[:], in_=r128(d_wv[:, half * 256:(half + 1) * 256]))
                w8s.append(w8)
            for jc in range(NJC):
                jw = 128 if jc < NJC - 1 else NP - 128 * (NJC - 1)
                ci = 2 if jc == NJC - 1 else jc // 4
                co = jc * 128 - IC_NP[ci][0]
                pss = [ppsv.tile([128, 256], F32, tag=f"ps_v{half}", name=f"ps_v{half}")
                       for half in range(2)]
                for kp in range(KD // 2):
                    for half in range(2):
                        nc.tensor.matmul(pss[half][:jw, :],
                                         ln1c[ci][:, 2 * kp:2 * kp + 2, co:co + jw],
                                         w8s[half][:, 2 * kp:2 * kp + 2, :],
                                         start=(kp == 0), stop=(kp == KD // 2 - 1),
                                         perf_mode=PM.DoubleRow)
                for half in range(2):
                    nc.vector.tensor_scalar_mul(
                        v1[:jw, jc, half * 4:(half + 1) * 4, 0:CC],
                        pss[half][:jw, :].rearrange("p (h c) -> p h c", c=CC),
                        1.0 / SW_QKV)
        with (
            tc.tile_pool(name="pwcma", bufs=2) as pwcma,
            tc.tile_pool(name="pps1", bufs=2, space="PSUM") as pps1,
        ):
            for d_w, dest, use_act in ((d_wq, qT, False), (d_wk, kT, True)):
                for half in range(2):
                    w8 = pwcma.tile([128, KD, 256], FP8, tag="w_cma")
                    nc.sync.dma_start(out=w8[:],
                                      in_=r128(d_w[:, half * 256:(half + 1) * 256]))
                    for mh in range(2):
                        mc = half * 2 + mh
                        for i, (io, iw) in enumerate(IC_NP):
                            ps = pps1.tile([128, iw], F32, tag=f"ps_qk{i}",
                                           name=f"ps_qk{i}")
                            for kp in range(KD // 2):
                                nc.tensor.matmul(
                                    ps[:], w8[:, 2 * kp:2 * kp + 2, mh * 128:(mh + 1) * 128],
                                    ln1c[i][:, 2 * kp:2 * kp + 2, :],
                                    start=(kp == 0), stop=(kp == KD // 2 - 1),
                                    perf_mode=PM.DoubleRow)
                            if use_act:
                                nc.scalar.activation(dest[:, mc, io:io + iw],
                                                     ps[:], AF.Copy,
                                                     scale=1.0 / SW_QKV)
                            else:
                                nc.vector.tensor_scalar(
                                    dest[:, mc, io:io + iw], ps[:],
                                    1.0 / SW_QKV, qb_t[:, mc:mc + 1],
                                    ALU.mult, ALU.add)
        pln1.release()

        # Attention: simT[j,i] per head-pair over the NV visual query columns.
        pcat = tc.alloc_tile_pool(name="pcat", bufs=1, side="right")
        catR = pcat.tile([128, 4, NV], BF16, tag="catR")   # un-normalized
        catT = pcat.tile([128, 4, NV], FP8, tag="catT")    # catR * 8/den
        den8 = pcat.tile([HC, NV], F32, tag="den8")
        rden8 = pcat.tile([HC, NV], BF16, tag="rden8")
        w_co8 = pcat.tile([128, 4, D], FP8, tag="w_co8")
        nc.sync.dma_start(out=w_co8[:], in_=r128(d_wco))
        with (
            tc.tile_pool(name="pm16", bufs=4) as pm16,
            tc.tile_pool(name="ppt", bufs=2) as ppt,
            tc.tile_pool(name="pden", bufs=2) as pden,
        ):
            with (
                tc.tile_pool(name="psim", bufs=1, space="PSUM") as psim,
                tc.tile_pool(name="pav", bufs=1, space="PSUM") as pav,
            ):
                for p in range(4):
                    avs = [pav.tile([CC + 1, NV], F32, tag=f"ps_av{e}",
                                    name=f"ps_av{e}") for e in range(2)]
                    for jc in range(NJC):
                        jw = 128 if jc < NJC - 1 else NP - 128 * (NJC - 1)
                        m8s, pss = [], []
                        for e in range(2):
                            h = 2 * p + e
                            m8 = pm16.tile([128, NV], BF16, tag=f"m8_{e}",
                                           name=f"m8_{e}")
                            nc.sync.dma_start(
                                out=m8[:jw, :],
                                in_=d_mask[h, jc * 128:jc * 128 + jw, :])
                            m8s.append(m8)
                            pss.append(psim.tile([128, NV], F32,
                                                 tag=f"ps_sim{e}",
                                                 name=f"ps_sim{e}"))
                        for (io, iw) in IC_NV:
                            for e in range(2):
                                pr = e * 64
                                nc.tensor.matmul(
                                    pss[e][:jw, io:io + iw],
                                    kT[pr:pr + 64, p, jc * 128:jc * 128 + jw],
                                    qT[pr:pr + 64, p, io:io + iw],
                                    start=True, stop=True)
                        ptms = []
                        for e in range(2):
                            pt = ppt.tile([128, NV], BF16, tag=f"pt{e}",
                                          name=f"pt{e}")
                            nc.scalar.activation(pt[:jw, :], pss[e][:jw, :],
                                                 AF.Exp, scale=SCALE)
                            ptm = ppt.tile([128, NV], BF16, tag=f"ptm{e}",
                                           name=f"ptm{e}")
                            nc.vector.tensor_mul(ptm[:jw, :], pt[:jw, :],
                                                 m8s[e][:jw, :])
                            ptms.append(ptm)
                        for (io, iw) in IC_NV:
                            for e in range(2):
                                nc.tensor.matmul(
                                    avs[e][:, io:io + iw],
                                    v1[:jw, jc, 2 * p + e, :],
                                    ptms[e][:jw, io:io + iw],
                                    start=(jc == 0), stop=(jc == NJC - 1))
                    for e in range(2):
                        pr = e * 64
                        nc.vector.tensor_copy(catR[pr:pr + 64, p, :],
                                              avs[e][0:CC, :])
                        den_st = pden.tile([1, NV], F32, tag="den_st")
                        nc.vector.tensor_copy(den_st[:], avs[e][CC:CC + 1, :])
                        nc.sync.dma_start(out=den8[2 * p + e:2 * p + e + 1, :],
                                          in_=den_st[:])
            # batched reciprocal: rden = 8/den via exp(-ln(den)+ln8) on ACT
            dln = pden.tile([HC, NV], F32, tag="dln")
            nc.scalar.activation(dln[:], den8[:], AF.Ln)
            nc.scalar.activation(rden8[:], dln[:], AF.Exp, scale=-1.0, bias=ln8_t[:])
            with tc.tile_pool(name="pdbc", bufs=2, space="PSUM") as pdbc:
                for hc in range(4):
                    for (io, iw) in IC_NV:
                        pd = pdbc.tile([128, iw], F32, tag="pd")
                        nc.tensor.matmul(pd[:], sel1[:, hc, :], rden8[:, io:io + iw],
                                         start=True, stop=True)
                        nc.vector.tensor_mul(catT[:, hc, io:io + iw],
                                             catR[:, hc, io:io + iw], pd[:])

        # Output projection (tanh/2048-folded) + residual into hsT.
        with tc.tile_pool(name="pco", bufs=2, space="PSUM") as pco:
            for mc in range(KD):
                for (io, iw) in IC_NV:
                    ps = pco.tile([128, iw], F32, tag="ps_co")
                    for kp in range(2):
                        nc.tensor.matmul(ps[:], w_co8[:, 2 * kp:2 * kp + 2, mc * 128:(mc + 1) * 128],
                                         catT[:, 2 * kp:2 * kp + 2, io:io + iw],
                                         start=(kp == 0), stop=(kp == 1),
                                         perf_mode=PM.DoubleRow)
                    nc.scalar.activation(ps[:], ps[:], AF.Identity,
                                         bias=bco_t[:, mc:mc + 1])
                    nc.vector.scalar_tensor_tensor(
                        out=hsT[mc][:, io:io + iw], in0=ps[:],
                        scalar=1.0 / (SW_CO * CAT_S),
                        in1=xT[mc][:, io:io + iw],
                        op0=ALU.mult, op1=ALU.add)
        pv1.release()
        pqk.release()
        pcat.release()
        px.release()  # xT dead

        # ================= Phase 2: LN2 + GEGLU FFN (fp8) =====================
        pln2 = tc.alloc_tile_pool(name="pln2", bufs=1)
        ln2c = [pln2.tile([128, KD, w], FP8, tag=f"ln2c{i}")
                for i, (o, w) in enumerate(IC_NV)]
        _emit_layernorm2(tc, nc, hsT, ln2c, NV, ones_r, ones_row128, eps_t, "ln2")
        penc = tc.alloc_tile_pool(name="penc", bufs=1, side="right")
        enc_sb = penc.tile([128, KT, LTP], BF16, tag="enc_sb")
        kTa = penc.tile([128, KD, LTP], BF16, tag="kTa")
        v1a = penc.tile([LTP, HA, CA + 1], BF16, tag="v1a")
        wka = penc.tile([128, KT, INNER_A], BF16, tag="wka")
        wva = penc.tile([128, KT, INNER_A], BF16, tag="wva")
        pff = tc.alloc_tile_pool(name="pff", bufs=1, side="right")
        ffT = pff.tile([128, KF, NV], FP8, tag="ffT")    # 16*(a+b)*gelu
        pwf = tc.alloc_tile_pool(name="pwf", bufs=3)
        wf0 = None
        with (
            tc.tile_pool(name="pwg", bufs=4) as pwg,
            tc.tile_pool(name="p2s", bufs=3) as p2s,
            tc.tile_pool(name="p2ps", bufs=2, space="PSUM") as p2ps,
        ):
            for m in range(KF):
                if m == 20:
                    # queue phase-3 weight DMAs mid-FFN1 so they stream
                    # behind the wg traffic instead of after it
                    nc.sync.dma_start(out=enc_sb[:], in_=r128(d_encT))
                    nc.sync.dma_start(out=v1a[:, :, CA:CA + 1],
                                      in_=d_vones.unsqueeze(2))
                    nc.sync.dma_start(out=wka[:], in_=r128(d_wka))
                    nc.sync.dma_start(out=wva[:], in_=r128(d_wva))
                if m == 30:
                    wf0 = pwf.tile([128, KF, 256], FP8, tag="wf", name="wf0")
                    nc.sync.dma_start(out=wf0[:], in_=r128(d_wf[:, 0:256]))
                if m % 2 == 0:
                    wga = pwg.tile([128, KD, 256], FP8, tag="wga")
                    nc.sync.dma_start(out=wga[:], in_=r128(d_wg[:, m * 128:(m + 2) * 128]))
                    wgg = pwg.tile([128, KD, 256], FP8, tag="wgg")
                    nc.sync.dma_start(out=wgg[:],
                                      in_=r128(d_wg[:, DFF + m * 128:DFF + (m + 2) * 128]))
                mo = (m % 2) * 128
                ps_a = p2ps.tile([128, NV], F32, tag="ps_a")
                ps_g = p2ps.tile([128, NV], F32, tag="ps_g")
                for kp in range(KD // 2):
                    for i, (io, iw) in enumerate(IC_NV):
                        nc.tensor.matmul(ps_a[:, io:io + iw], wga[:, 2 * kp:2 * kp + 2, mo:mo + 128],
                                         ln2c[i][:, 2 * kp:2 * kp + 2, :],
                                         start=(kp == 0), stop=(kp == KD // 2 - 1),
                                         perf_mode=PM.DoubleRow)
                    for i, (io, iw) in enumerate(IC_NV):
                        nc.tensor.matmul(ps_g[:, io:io + iw], wgg[:, 2 * kp:2 * kp + 2, mo:mo + 128],
                                         ln2c[i][:, 2 * kp:2 * kp + 2, :],
                                         start=(kp == 0), stop=(kp == KD // 2 - 1),
                                         perf_mode=PM.DoubleRow)
                gelu_sb = p2s.tile([128, NV], BF16, tag="gelu_sb")
                nc.scalar.activation(gelu_sb[:], ps_g[:], GELU_AF,
                                     scale=1.0 / SW_G, bias=bgg_t[:, m:m + 1])
                nc.vector.scalar_tensor_tensor(
                    out=ffT[:, m, :], in0=ps_a[:], scalar=bga_t[:, m:m + 1],
                    in1=gelu_sb[:], op0=ALU.add, op1=ALU.mult)
        # ffout (tanh/2048-folded) + residual in place.
        with (
            tc.tile_pool(name="pfps", bufs=2, space="PSUM") as pfps,
        ):
            for mc in range(KD):
                if mc == 0:
                    wf = wf0
                elif mc % 2 == 0:
                    wf = pwf.tile([128, KF, 256], FP8, tag="wf")
                    nc.sync.dma_start(out=wf[:], in_=r128(d_wf[:, mc * 128:(mc + 2) * 128]))
                mo = (mc % 2) * 128
                pss = [pfps.tile([128, iw], F32, tag=f"ps_f{i}")
                       for i, (io, iw) in enumerate(IC_NV)]
                for kp in range(KF // 2):
                    for i, (io, iw) in enumerate(IC_NV):
                        nc.tensor.matmul(pss[i][:], wf[:, 2 * kp:2 * kp + 2, mo:mo + 128],
                                         ffT[:, 2 * kp:2 * kp + 2, io:io + iw],
                                         start=(kp == 0), stop=(kp == KF // 2 - 1),
                                         perf_mode=PM.DoubleRow)
                for i, (io, iw) in enumerate(IC_NV):
                    nc.scalar.activation(pss[i][:], pss[i][:], AF.Identity,
                                         bias=bf_t[:, mc:mc + 1])
                    nc.vector.scalar_tensor_tensor(
                        out=hsT[mc][:, io:io + iw], in0=pss[i][:],
                        scalar=1.0 / (SW_F * SW_G),
                        in1=hsT[mc][:, io:io + iw],
                        op0=ALU.mult, op1=ALU.add)
        pwf.release()
        pln2.release()
        pff.release()

        # ============== Phase 3: enc projections, LN3, q3, cross-attn =========
        # enc k/v projections are independent of LN3 (weights were DMA'd
        # during FFN1) — emit first so they overlap the LN3 stats.
        with tc.tile_pool(name="ppenc", bufs=2, space="PSUM") as ppenc:
            for mc in range(KD):
                ps = ppenc.tile([128, LTP], F32, tag="ps_enc")
                for kc in range(KT):
                    nc.tensor.matmul(ps[:], wka[:, kc, mc * 128:(mc + 1) * 128],
                                     enc_sb[:, kc, :], start=(kc == 0), stop=(kc == KT - 1))
                nc.vector.tensor_copy(kTa[:, mc, :], ps[:])
            for (co, cw) in DC_D:
                ps = ppenc.tile([LTP, 512], F32, tag="ps_encv")
                for kc in range(KT):
                    nc.tensor.matmul(ps[:, :cw], enc_sb[:, kc, :], wva[:, kc, co:co + cw],
                                     start=(kc == 0), stop=(kc == KT - 1))
                nc.vector.tensor_copy(v1a[:, co // CA:(co + cw) // CA, 0:CA],
                                      ps[:, :cw].rearrange("p (h c) -> p h c", c=CA))

        pln3 = tc.alloc_tile_pool(name="pln3", bufs=1, side="right")
        ln3c = [pln3.tile([128, KD, w], FP8, tag=f"ln3c{i}")
                for i, (o, w) in enumerate(IC_NV)]
        _emit_layernorm2(tc, nc, hsT, ln3c, NV, ones_r, ones_row128, eps_t, "ln3")
        res.release()  # hsT dead

        pq3 = tc.alloc_tile_pool(name="pq3", bufs=1)
        qTa = pq3.tile([128, KD, NV], BF16, tag="qTa")
        with (
            tc.tile_pool(name="pwqa", bufs=3) as pwqa,
            tc.tile_pool(name="pp3", bufs=2, space="PSUM") as pp3,
        ):
            for half in range(5):
                wqa = pwqa.tile([128, KD, 256], FP8, tag="wqa")
                nc.sync.dma_start(out=wqa[:],
                                  in_=r128(d_wqa[:, half * 256:(half + 1) * 256]))
                for mh in range(2):
                    mc = half * 2 + mh
                    pss = [pp3.tile([128, iw], F32, tag=f"ps_p3{i}")
                           for i, (io, iw) in enumerate(IC_NV)]
                    for kp in range(KD // 2):
                        for i, (io, iw) in enumerate(IC_NV):
                            nc.tensor.matmul(pss[i][:], wqa[:, 2 * kp:2 * kp + 2, mh * 128:(mh + 1) * 128],
                                             ln3c[i][:, 2 * kp:2 * kp + 2, :],
                                             start=(kp == 0), stop=(kp == KD // 2 - 1),
                                             perf_mode=PM.DoubleRow)
                    for i, (io, iw) in enumerate(IC_NV):
                        nc.vector.tensor_scalar(qTa[:, mc, io:io + iw], pss[i][:],
                                                1.0 / SW_QKV, qab_t[:, mc:mc + 1],
                                                ALU.mult, ALU.add)
        pln3.release()

        # cross-attention with 2-group denominator + 2-pass output projection
        pcat3 = tc.alloc_tile_pool(name="pcat3", bufs=1, side="right")
        catTa = [pcat3.tile([128, NV], BF16, tag=f"catTa{hc}") for hc in range(KD)]
        catB = [pcat3.tile([128, NV], BF16, tag=f"catB{hc}") for hc in range(KD)]
        den_g = [pcat3.tile([KD, NV], F32, tag=f"den_g{g}") for g in range(2)]
        rden_g = [pcat3.tile([KD, NV], BF16, tag=f"rden_g{g}") for g in range(2)]
        popart = tc.alloc_tile_pool(name="popart", bufs=1)
        opart = [popart.tile([128, D], F32, tag=f"opart{it}") for it in range(8)]
        pwoa = tc.alloc_tile_pool(name="pwoa", bufs=1)
        woa = pwoa.tile([128, KD, D], BF16, tag="woa")
        nc.sync.dma_start(out=woa[:], in_=r128(d_woa))

        with (
            tc.tile_pool(name="p3s", bufs=3) as p3s,
            tc.tile_pool(name="pden3", bufs=2) as pden3,
            tc.tile_pool(name="psa", bufs=1, space="PSUM") as psa,
            tc.tile_pool(name="pava", bufs=1, space="PSUM") as pava,
            tc.tile_pool(name="pdbca", bufs=1, space="PSUM") as pdbca,
            tc.tile_pool(name="poo", bufs=1, space="PSUM") as poo,
            tc.tile_pool(name="po", bufs=3) as po,
        ):
            ptas = {}

            def emit_sim3(h):
                pr = (h % 2) * 64
                hc = h // 2
                pta = p3s.tile([LTP, NV], BF16, tag=f"pta{h % 3}")
                for i, (io, iw) in enumerate(IC_NV):
                    ps_s = psa.tile([LTP, 512], F32, tag=f"ps_sa{i}")
                    nc.tensor.matmul(ps_s[:, :iw], kTa[pr:pr + 64, hc, :],
                                     qTa[pr:pr + 64, hc, io:io + iw],
                                     start=True, stop=True)
                    nc.scalar.activation(pta[:, io:io + iw], ps_s[:, :iw], AF.Exp,
                                         scale=SCALE)
                ptas[h] = pta

            def emit_av3(h):
                pr = (h % 2) * 64
                hc = h // 2
                g = h // 10
                pta = ptas.pop(h)
                av = pava.tile([CA + 1, NV], F32, tag="ps_ava")
                for (io, iw) in IC_NV:
                    nc.tensor.matmul(av[:, io:io + iw], v1a[:, h, :],
                                     pta[:, io:io + iw], start=True, stop=True)
                if h % 2 == 0:
                    nc.vector.tensor_copy(catTa[hc][0:64, :], av[0:CA, :])
                else:
                    nc.scalar.activation(catTa[hc][64:128, :], av[0:CA, :], AF.Copy)
                den_st = pden3.tile([1, NV], F32, tag="den_st3")
                nc.vector.tensor_copy(den_st[:], av[CA:CA + 1, :])
                nc.sync.dma_start(out=den_g[g][h - 10 * g:h - 10 * g + 1, :],
                                  in_=den_st[:])

            def emit_group_norm(g):
                # rden for heads 10g..10g+9, then catB for hc 5g..5g+4;
                # sel3g row index is the within-group head index.
                dln = pden3.tile([KD, NV], F32, tag=f"dln3{g}")
                nc.scalar.activation(dln[:], den_g[g][:], AF.Ln)
                nc.scalar.activation(rden_g[g][:], dln[:], AF.Exp, scale=-1.0)
                for hc in range(5 * g, 5 * g + 5):
                    for (io, iw) in IC_NV:
                        pd = pdbca.tile([128, iw], F32, tag="pda")
                        nc.tensor.matmul(pd[:], sel3g[:, hc - 5 * g, :],
                                         rden_g[g][:, io:io + iw],
                                         start=True, stop=True)
                        nc.vector.tensor_mul(catB[hc][:, io:io + iw],
                                             catTa[hc][:, io:io + iw], pd[:])

            def emit_oproj_pass(kcs, first):
                for it in range(NV // 128):
                    for i, (dco, dcw) in enumerate(DC_D):
                        ps = poo.tile([128, dcw], F32, tag=f"ps_oo{i}")
                        for j, kc in enumerate(kcs):
                            nc.tensor.matmul(ps[:], catB[kc][:, it * 128:(it + 1) * 128],
                                             woa[:, kc, dco:dco + dcw],
                                             start=(j == 0), stop=(j == len(kcs) - 1))
                        if first:
                            nc.vector.tensor_add(opart[it][:, dco:dco + dcw], ps[:],
                                                 boa_b[:, dco:dco + dcw])
                        else:
                            o_sb = po.tile([128, dcw], F32, tag="o_sb")
                            nc.vector.tensor_add(o_sb[:], ps[:],
                                                 opart[it][:, dco:dco + dcw])
                            nc.sync.dma_start(
                                out=d_out[it * 128:(it + 1) * 128, dco:dco + dcw],
                                in_=o_sb[:])

            sel3g = cst.tile([KD, 5, 128], BF16, tag="sel3g")
            nc.vector.tensor_copy(sel3g[:], sel3[0:KD, 0:5, :])

            emit_sim3(0)
            emit_sim3(1)
            for h in range(2, 10):
                emit_sim3(h)
                emit_av3(h - 2)
            emit_av3(8)
            emit_av3(9)
            emit_group_norm(0)
            emit_sim3(10)
            emit_sim3(11)
            emit_oproj_pass(list(range(5)), True)
            for h in range(12, 20):
                emit_sim3(h)
                emit_av3(h - 2)
            emit_av3(18)
            emit_av3(19)
            emit_group_norm(1)
            emit_oproj_pass(list(range(5, KD)), False)
        pcat3.release()
        pwoa.release()
        popart.release()
        pq3.release()
        penc.release()
        cst.release()

    nc.compile()
    return nc


_CACHE = {}


def _get_nc():
    if "nc" not in _CACHE:
        _CACHE["nc"] = build_nc()
    return _CACHE["nc"]


def _sel(nh, npairs):
    bf16 = ml_dtypes.bfloat16
    s = np.zeros((nh, npairs, 128), dtype=bf16)
    for hc in range(npairs):
        s[2 * hc, hc, 0:64] = 1
        s[2 * hc + 1, hc, 64:128] = 1
    return s


def prepare_in_maps(inputs):
    f32 = np.float32
    bf16 = ml_dtypes.bfloat16
    fp8 = ml_dtypes.float8_e4m3
    hidT = np.ascontiguousarray(inputs["hidden_states"].transpose(0, 2, 1)).astype(bf16)
    objT = np.ascontiguousarray(inputs["object_embeddings"].transpose(0, 2, 1)).astype(bf16)
    encT = np.zeros((B, DTXT, LTP), dtype=f32)
    encT[:, :, :LTXT] = inputs["encoder_hidden_states"].transpose(0, 2, 1)
    masks = inputs["object_attention_masks"]
    mask8 = np.zeros((B, HC, NP, NV), dtype=bf16)
    mask8[:, :, :N, :] = (masks.transpose(0, 1, 3, 2)[:, :, :, :NV] > 0)

    ta = float(np.tanh(inputs["alpha_attn"]))
    td = float(np.tanh(inputs["alpha_dense"]))
    g1 = np.asarray(inputs["norm1_g"], dtype=f32)
    b1 = np.asarray(inputs["norm1_b"], dtype=f32)
    g2 = np.asarray(inputs["norm2_g"], dtype=f32)
    b2 = np.asarray(inputs["norm2_b"], dtype=f32)
    g3 = np.asarray(inputs["norm3_g"], dtype=f32)
    b3 = np.asarray(inputs["norm3_b"], dtype=f32)

    w_q = np.asarray(inputs["cma_q_w"], dtype=f32)
    w_k = np.asarray(inputs["cma_k_w"], dtype=f32)
    w_v = np.asarray(inputs["cma_v_w"], dtype=f32)
    w_co = np.asarray(inputs["cma_out_w"], dtype=f32)
    # beta folds: q bias explicit; k bias cancels in softmax; v bias shifts
    # every attention output by vb (softmax weights sum to 1) -> co bias.
    qb = b1 @ w_q                       # [512]
    vb = b1 @ w_v                       # [512]
    bco = np.asarray(inputs["cma_out_b"], dtype=f32) + vb @ w_co
    w_geglu = np.asarray(inputs["geglu_w"], dtype=f32)
    bg = np.asarray(inputs["geglu_b"], dtype=f32) + b2 @ w_geglu
    w_qa = np.asarray(inputs["attn_q_w"], dtype=f32)
    qab = b3 @ w_qa                     # [1280]

    w_co8 = (w_co * (ta * SW_CO)).astype(fp8)
    bco_cols = np.ascontiguousarray(
        (bco * (ta * SW_CO * CAT_S)).reshape(KD, 128).T)
    w_f8 = (np.asarray(inputs["ffout_w"]) * (td * SW_F)).astype(fp8)
    bf_cols = np.ascontiguousarray(
        (np.asarray(inputs["ffout_b"], dtype=f32) * (td * SW_F * SW_G))
        .reshape(KD, 128).T)
    shared = {
        "w_lin": np.asarray(inputs["linear_w"]).astype(bf16),
        "b_lin": np.ascontiguousarray(inputs["linear_b"], dtype=f32),
        "w_q8": (w_q * g1[:, None] * SW_QKV).astype(fp8),
        "w_k8": (w_k * g1[:, None] * SW_QKV).astype(fp8),
        "w_v8": (w_v * g1[:, None] * SW_QKV).astype(fp8),
        "qb_cols": np.ascontiguousarray(qb.reshape(4, 128).T, dtype=f32),
        "w_co8": w_co8, "bco_cols": bco_cols,
        "w_g8": (w_geglu * g2[:, None] * SW_G).astype(fp8),
        "bg_a16": (bg[:DFF] * SW_G).astype(f32),
        "bg_g": bg[DFF:].astype(f32),
        "w_f8": w_f8, "bf_cols": bf_cols,
        "w_qa8": (w_qa * g3[:, None]).astype(bf16),
        "qab_cols": np.ascontiguousarray(qab.reshape(KD, 128).T, dtype=f32),
        "w_ka": np.asarray(inputs["attn_k_w"]).astype(bf16),
        "w_va": np.asarray(inputs["attn_v_w"]).astype(bf16),
        "w_oa": np.asarray(inputs["attn_out_w"]).astype(bf16),
        "b_oa": np.ascontiguousarray(inputs["attn_out_b"], dtype=f32),
        "vones": np.concatenate([np.ones((LTXT, HA)),
                                 np.zeros((LTP - LTXT, HA))], axis=0).astype(bf16),
        "sel1": _sel(HC, 4), "sel3": _sel(HA, KD),
    }
    in_maps = []
    for b in range(B):
        m = dict(shared)
        m["hidT"] = hidT[b]
        m["objT"] = objT[b]
        m["encT"] = encT[b].astype(bf16)
        m["mask8"] = np.ascontiguousarray(mask8[b])
        in_maps.append(m)
    return in_maps


def run(inputs, trace=False):
    nc = _get_nc()
    in_maps = prepare_in_maps(inputs)
    res = run_bass_kernel_spmd(nc, in_maps, core_ids=list(range(B)), trace=trace)
    out = np.stack([res.results[b]["out"] for b in range(B)], axis=0)
    return out, res


def kernel(**inputs):
    out, _ = run(inputs, trace=False)
    return out


# revision 22
# speedup vs baseline: 1.1549x; 1.0262x over previous
"""Trainium2 Bass kernel for nn_CustomAttnProcessor (dense transformer block).

Data-parallel over batch B=8 across 8 NeuronCores; one batch element per core.

v2 layout (channel-major activations: [feature_partition, token_free]):
  - LN gammas folded into downstream weights host-side; betas folded into
    downstream biases (k-side betas drop out of softmax). On-device LN only
    computes x_hat = (x - mu) * rstd via 2 broadcast matmuls per token chunk.
  - LN stats (mu, E[x^2]) col-packed at PSUM partitions 0/32 (2x concurrency).
  - QKV / FFN2 loops are kp-outer so LDWEIGHTS amortizes over token chunks.
  - attn1 heads processed in even/odd pairs at PE row groups 0/64.
  - attn3: per-head fat exp, denominators normalized in 2 head groups so the
    final projection's contraction runs as 2 passes (kc 0-4 / 5-9), the first
    overlapping the second half of attention.
  - biases applied via ACT Identity into PSUM / DVE tensor_scalar, not rank-1
    matmuls.
"""

import math
import os
import sys

import numpy as np
import ml_dtypes

sys.path.insert(0, "/opt/trn_rl_repo")

import concourse.bass as bass
import concourse.tile as tile
from concourse import bacc, mybir
from concourse.bass_utils import run_bass_kernel_spmd

F32 = mybir.dt.float32
F32R = mybir.dt.float32r
BF16 = mybir.dt.bfloat16
FP8 = mybir.dt.float8e4
AF = mybir.ActivationFunctionType
ALU = mybir.AluOpType
PM = mybir.MatmulPerfMode
GELU_AF = AF.Tanh if os.environ.get("SIM_SAFE_GELU") else AF.Gelu

B = 8
NV = 1024          # visual tokens (the only query columns computed)
NOBJ = 30
N = NV + NOBJ      # 1054
NP = 1056          # padded token count (keys)
NJC = 9            # key-dim 128-chunks over NP (last chunk = 32 rows)
D = 1280
KD = D // 128      # 10
DTXT = 768
KT = DTXT // 128   # 6
LTXT = 77
LTP = 78           # padded
HC, CC = 8, 64     # masked self-attention heads
HA, CA = 20, 64    # cross-attention heads
INNER_C = HC * CC  # 512
INNER_A = HA * CA  # 1280
DFF = 4 * D        # 5120
KF = DFF // 128    # 40
EPS = 1e-5
SCALE = CC ** -0.5  # 0.125

SW_QKV = 256.0     # w_q/k/v, w_qa
SW_CO = 2048.0     # w_co (tanh-folded)
SW_G = 16.0        # w_geglu (also leaves ffT scaled x16 for fp8 range)
SW_F = 2048.0      # w_ffout (tanh-folded)
CAT_S = 8.0        # catT = 8*attn (keeps fp8 out of denormals)

IC_NP = [(0, 512), (512, 512), (1024, 32)]   # token chunks for 1056
IC_NV = [(0, 512), (512, 512)]               # token chunks for 1024
DC_D = [(0, 512), (512, 512), (1024, 256)]   # feature chunks for 1280


def _emit_layernorm2(tc, nc, xs, outs, n_tok, ones_r, ones_row128, eps_t,
                     st_name):
    """x_hat = (x - mu) * rstd over the partition (feature) axis.

    xs:   list of KD tiles [128, >=n_tok] (only [:, :n_tok] read)
    outs: list of per-chunk tiles [128, KD, w] (fp8/bf16)
    """
    chunks = [(o, w) for (o, w) in IC_NP if o < n_tok]
    nch = len(chunks)
    inv_d = 1.0 / float(KD * 128)
    with (
        tc.tile_pool(name=f"{st_name}_rows", bufs=1) as rows,
        tc.tile_pool(name=f"{st_name}_t1", bufs=3) as t1p,
    ):
        mu3 = rows.tile([1, nch, 512], F32, tag="mu3")
        ex3 = rows.tile([1, nch, 512], F32, tag="ex3")
        var3 = rows.tile([1, nch, 512], F32, tag="var3")
        rs3 = rows.tile([1, nch, 512], F32R, tag="rs3")
        murs3 = rows.tile([1, nch, 512], F32R, tag="murs3")
        with (
            tc.tile_pool(name=f"{st_name}_st", bufs=1, space="PSUM") as stp,
            tc.tile_pool(name=f"{st_name}_sq", bufs=2) as sqp,
        ):
            st_mu = stp.tile([1, nch, 512], F32, tag="st_mu")
            st_ex = stp.tile([1, nch, 512], F32, tag="st_ex")
            for kc in range(KD):
                sq = sqp.tile([128, n_tok], BF16, tag="ln_sq")
                nc.scalar.activation(sq[:], xs[kc][:, 0:n_tok], AF.Square)
                for i, (o, w) in enumerate(chunks):
                    nc.tensor.matmul(st_mu[:, i, :w], ones_r[:],
                                     xs[kc][:, o:o + w],
                                     start=(kc == 0), stop=(kc == KD - 1))
                    nc.tensor.matmul(st_ex[:, i, :w], ones_r[:], sq[:, o:o + w],
                                     start=(kc == 0), stop=(kc == KD - 1))
            nc.vector.tensor_scalar_mul(mu3[:], st_mu[:], inv_d)
            nc.vector.tensor_scalar_mul(ex3[:], st_ex[:], inv_d)
        nc.vector.tensor_mul(var3[:], mu3[:], mu3[:])
        nc.vector.tensor_sub(var3[:], ex3[:], var3[:])
        # rstd = exp(-0.5*ln(var+eps))
        nc.scalar.activation(ex3[:], var3[:], AF.Ln, bias=eps_t[:])
        nc.scalar.activation(rs3[:], ex3[:], AF.Exp, scale=-0.5)
        nc.vector.tensor_mul(murs3[:], mu3[:], rs3[:].bitcast(F32))
        with (
            tc.tile_pool(name=f"{st_name}_bc", bufs=2, space="PSUM") as bcp,
            tc.tile_pool(name=f"{st_name}_bcs", bufs=2) as bcsp,
        ):
            for i, (o, w) in enumerate(chunks):
                bcr = bcp.tile([128, 512], F32, tag="bcr")
                bcm = bcp.tile([128, 512], F32, tag="bcm")
                nc.tensor.matmul(bcr[:, :w], ones_row128[:], rs3[:, i, :w],
                                 start=True, stop=True)
                nc.tensor.matmul(bcm[:, :w], ones_row128[:],
                                 murs3[:, i, :w], start=True, stop=True)
                # stage broadcasts to SBUF bf16 so the apply runs at DVE 2x
                bcr_s = bcsp.tile([128, 512], BF16, tag="bcr_s")
                bcm_s = bcsp.tile([128, 512], BF16, tag="bcm_s")
                nc.scalar.copy(bcr_s[:, :w], bcr[:, :w])
                nc.scalar.copy(bcm_s[:, :w], bcm[:, :w])
                for kc in range(KD):
                    eng = nc.vector if kc < 7 else nc.gpsimd
                    t1 = t1p.tile([128, 512], BF16, tag="ln_t1")
                    eng.tensor_mul(t1[:, :w], xs[kc][:, o:o + w], bcr_s[:, :w])
                    eng.tensor_sub(outs[i][:, kc, :w], t1[:, :w], bcm_s[:, :w])


def build_nc():
    nc = bacc.Bacc("TRN2", target_bir_lowering=False, debug=False, num_devices=B)

    # ---- DRAM I/O (per core) ----
    d_hidT = nc.dram_tensor("hidT", [D, NV], BF16, kind="ExternalInput").ap()
    d_objT = nc.dram_tensor("objT", [DTXT, NOBJ], BF16, kind="ExternalInput").ap()
    d_encT = nc.dram_tensor("encT", [DTXT, LTP], BF16, kind="ExternalInput").ap()
    d_mask = nc.dram_tensor("mask8", [HC, NP, NV], BF16, kind="ExternalInput").ap()
    d_wlin = nc.dram_tensor("w_lin", [DTXT, D], BF16, kind="ExternalInput").ap()
    d_blin = nc.dram_tensor("b_lin", [D], F32, kind="ExternalInput").ap()
    d_wq = nc.dram_tensor("w_q8", [D, INNER_C], FP8, kind="ExternalInput").ap()
    d_wk = nc.dram_tensor("w_k8", [D, INNER_C], FP8, kind="ExternalInput").ap()
    d_wv = nc.dram_tensor("w_v8", [D, INNER_C], FP8, kind="ExternalInput").ap()
    d_qb = nc.dram_tensor("qb_cols", [128, 4], F32, kind="ExternalInput").ap()
    d_wco = nc.dram_tensor("w_co8", [INNER_C, D], FP8, kind="ExternalInput").ap()
    d_bco = nc.dram_tensor("bco_cols", [128, KD], F32, kind="ExternalInput").ap()
    d_wg = nc.dram_tensor("w_g8", [D, 2 * DFF], FP8, kind="ExternalInput").ap()
    d_bga = nc.dram_tensor("bg_a16", [DFF], F32, kind="ExternalInput").ap()
    d_bgg = nc.dram_tensor("bg_g", [DFF], F32, kind="ExternalInput").ap()
    d_wf = nc.dram_tensor("w_f8", [DFF, D], FP8, kind="ExternalInput").ap()
    d_bf = nc.dram_tensor("bf_cols", [128, KD], F32, kind="ExternalInput").ap()
    d_wqa = nc.dram_tensor("w_qa8", [D, INNER_A], BF16, kind="ExternalInput").ap()
    d_qab = nc.dram_tensor("qab_cols", [128, KD], F32, kind="ExternalInput").ap()
    d_wka = nc.dram_tensor("w_ka", [DTXT, INNER_A], BF16, kind="ExternalInput").ap()
    d_wva = nc.dram_tensor("w_va", [DTXT, INNER_A], BF16, kind="ExternalInput").ap()
    d_woa = nc.dram_tensor("w_oa", [INNER_A, D], BF16, kind="ExternalInput").ap()
    d_boa = nc.dram_tensor("b_oa", [D], F32, kind="ExternalInput").ap()
    d_vones = nc.dram_tensor("vones", [LTP, HA], BF16, kind="ExternalInput").ap()
    d_sel1 = nc.dram_tensor("sel1", [HC, 4, 128], BF16, kind="ExternalInput").ap()
    d_sel3 = nc.dram_tensor("sel3", [HA, KD, 128], BF16, kind="ExternalInput").ap()
    d_out = nc.dram_tensor("out", [NV, D], F32, kind="ExternalOutput").ap()

    r128 = lambda ap: ap.rearrange("(kc p) n -> p kc n", p=128)
    LN8 = math.log(CAT_S)

    with tile.TileContext(nc) as tc, \
            nc.allow_low_precision(reason="fp8/bf16 rounding is intentional"):
        cst = tc.alloc_tile_pool(name="cst", bufs=1)
        ones_f = cst.tile([128, 128], F32, tag="ones_f")
        nc.vector.memset(ones_f[:], 1.0)
        ones_r = cst.tile([128, 1], BF16, tag="ones_r")
        nc.vector.tensor_copy(ones_r[:], ones_f[:, 0:1])
        ones_row128 = cst.tile([1, 128], F32R, tag="ones_row128")
        nc.vector.tensor_copy(ones_row128[:], ones_f[0:1, :])
        zeros2 = cst.tile([128, 2], BF16, tag="zeros2")
        nc.vector.memset(zeros2[:], 0.0)
        eps_t = cst.tile([1, 1], F32, tag="eps_t")
        nc.vector.memset(eps_t[:], EPS)
        ln8_t = cst.tile([HC, 1], F32, tag="ln8_t")
        nc.vector.memset(ln8_t[:], LN8)
        sel1 = cst.tile([HC, 4, 128], BF16, tag="sel1")
        nc.sync.dma_start(out=sel1[:], in_=d_sel1)
        sel3 = cst.tile([HA, KD, 128], BF16, tag="sel3")
        nc.sync.dma_start(out=sel3[:], in_=d_sel3)
        blin_t = cst.tile([128, KD], F32, tag="blin")
        nc.sync.dma_start(out=blin_t[:], in_=d_blin.rearrange("(kc p) -> p kc", p=128))
        qb_t = cst.tile([128, 4], F32, tag="qb_t")
        nc.sync.dma_start(out=qb_t[:], in_=d_qb)
        bco_t = cst.tile([128, KD], F32, tag="bco_t")
        nc.sync.dma_start(out=bco_t[:], in_=d_bco)
        bga_t = cst.tile([128, KF], F32, tag="bga")
        nc.sync.dma_start(out=bga_t[:], in_=d_bga.rearrange("(kc p) -> p kc", p=128))
        bgg_t = cst.tile([128, KF], F32, tag="bgg")
        nc.sync.dma_start(out=bgg_t[:], in_=d_bgg.rearrange("(kc p) -> p kc", p=128))
        bf_t = cst.tile([128, KD], F32, tag="bf_t")
        nc.sync.dma_start(out=bf_t[:], in_=d_bf)
        qab_t = cst.tile([128, KD], F32, tag="qab_t")
        nc.sync.dma_start(out=qab_t[:], in_=d_qab)
        boa_b = cst.tile([128, D], F32, tag="boa_b")
        nc.sync.dma_start(out=boa_b[:], in_=bass.AP(
            tensor=d_boa.tensor, offset=d_boa.offset, ap=[[0, 128]] + d_boa.ap))

        probe_i = cst.tile([8, 512], F32, tag="probe_i")
        nc.vector.memset(probe_i[:], 2.0)
        probe_o = cst.tile([8, 512], F32, tag="probe_o")
        nc.vector.reciprocal(probe_o[:], probe_i[:])

        res = tc.alloc_tile_pool(name="res", bufs=1)  # hsT per-mc: phases 1-3
        hsT = [res.tile([128, NV], F32R, tag=f"hsT{mc}") for mc in range(KD)]

        # ================= Phase 1: concat + LN1 + masked self-attention ======
        px = tc.alloc_tile_pool(name="px", bufs=1)
        obj_sb = px.tile([128, KT, NOBJ], BF16, tag="obj_sb")
        nc.sync.dma_start(out=obj_sb[:], in_=r128(d_objT))
        xT = [px.tile([128, NP], F32R, tag=f"xT{kc}") for kc in range(KD)]
        with (
            tc.tile_pool(name="pwlin", bufs=1) as pwlin,
            tc.tile_pool(name="pps0", bufs=2, space="PSUM") as pps0,
        ):
            wlin = pwlin.tile([128, KT, D], BF16, tag="wlin")
            for kc in range(KT):
                nc.sync.dma_start(out=wlin[:, kc, :], in_=r128(d_wlin)[:, kc, :])
            for mc in range(KD):
                nc.sync.dma_start(out=xT[mc][:, 0:NV], in_=r128(d_hidT)[:, mc, :])
                ps = pps0.tile([128, NOBJ], F32, tag="ps_obj")
                for kc in range(KT):
                    nc.tensor.matmul(ps[:], wlin[:, kc, mc * 128:(mc + 1) * 128],
                                     obj_sb[:, kc, :], start=(kc == 0),
                                     stop=(kc == KT - 1))
                nc.scalar.activation(xT[mc][:, NV:N], ps[:], AF.Identity,
                                     bias=blin_t[:, mc:mc + 1])
                nc.vector.tensor_copy(xT[mc][:, N:NP], zeros2[:])

        pln1 = tc.alloc_tile_pool(name="pln1", bufs=1, side="right")
        ln1c = [pln1.tile([128, KD, w], FP8, tag=f"ln1c{i}")
                for i, (o, w) in enumerate(IC_NP)]
        _emit_layernorm2(tc, nc, xT, ln1c, NP, ones_r, ones_row128, eps_t, "ln1")

        pqk = tc.alloc_tile_pool(name="pqk", bufs=1)
        pv1 = tc.alloc_tile_pool(name="pv1", bufs=1)
        qT = pqk.tile([128, 4, NP], BF16, tag="qT")
        kT = pqk.tile([128, 4, NP], BF16, tag="kT")
        v1 = pv1.tile([128, NJC, HC, CC + 1], BF16, tag="v1")
        nc.vector.memset(v1[:, :, :, CC:CC + 1], 1.0)
        with (
            tc.tile_pool(name="pwcv", bufs=2) as pwcv,
            tc.tile_pool(name="ppsv", bufs=2, space="PSUM") as ppsv,
        ):
            w8s = []
            for half in range(2):
                w8 = pwcv.tile([128, KD, 256], FP8, tag="w_cv")
                nc.sync.dma_start(out=w8[:], in_=r128(d_wv[:, half * 256:(half + 1) * 256]))
                w8s.append(w8)
            for jc in range(NJC):
                jw = 128 if jc < NJC - 1 else NP - 128 * (NJC - 1)
                ci = 2 if jc == NJC - 1 else jc // 4
                co = jc * 128 - IC_NP[ci][0]
                pss = [ppsv.tile([128, 256], F32, tag=f"ps_v{half}", name=f"ps_v{half}")
                       for half in range(2)]
                for kp in range(KD // 2):
                    for half in range(2):
                        nc.tensor.matmul(pss[half][:jw, :],
                                         ln1c[ci][:, 2 * kp:2 * kp + 2, co:co + jw],
                                         w8s[half][:, 2 * kp:2 * kp + 2, :],
                                         start=(kp == 0), stop=(kp == KD // 2 - 1),
                                         perf_mode=PM.DoubleRow)
                for half in range(2):
                    nc.vector.tensor_scalar_mul(
                        v1[:jw, jc, half * 4:(half + 1) * 4, 0:CC],
                        pss[half][:jw, :].rearrange("p (h c) -> p h c", c=CC),
                        1.0 / SW_QKV)
        with (
            tc.tile_pool(name="pwcma", bufs=2) as pwcma,
            tc.tile_pool(name="pps1", bufs=2, space="PSUM") as pps1,
        ):
            for d_w, dest, use_act in ((d_wq, qT, False), (d_wk, kT, True)):
                for half in range(2):
                    w8 = pwcma.tile([128, KD, 256], FP8, tag="w_cma")
                    nc.sync.dma_start(out=w8[:],
                                      in_=r128(d_w[:, half * 256:(half + 1) * 256]))
                    for mh in range(2):
                        mc = half * 2 + mh
                        for i, (io, iw) in enumerate(IC_NP):
                            ps = pps1.tile([128, iw], F32, tag=f"ps_qk{i}",
                                           name=f"ps_qk{i}")
                            for kp in range(KD // 2):
                                nc.tensor.matmul(
                                    ps[:], w8[:, 2 * kp:2 * kp + 2, mh * 128:(mh + 1) * 128],
                                    ln1c[i][:, 2 * kp:2 * kp + 2, :],
                                    start=(kp == 0), stop=(kp == KD // 2 - 1),
                                    perf_mode=PM.DoubleRow)
                            if use_act:
                                nc.scalar.activation(dest[:, mc, io:io + iw],
                                                     ps[:], AF.Copy,
                                                     scale=1.0 / SW_QKV)
                            else:
                                nc.vector.tensor_scalar(
                                    dest[:, mc, io:io + iw], ps[:],
                                    1.0 / SW_QKV, qb_t[:, mc:mc + 1],
                                    ALU.mult, ALU.add)
        pln1.release()

        # Attention: simT[j,i] per head-pair over the NV visual query columns.
        pcat = tc.alloc_tile_pool(name="pcat", bufs=1, side="right")
        catR = pcat.tile([128, 4, NV], BF16, tag="catR")   # un-normalized
        catT = pcat.tile([128, 4, NV], FP8, tag="catT")    # catR * 8/den
        den8 = pcat.tile([HC, NV], F32, tag="den8")
        rden8 = pcat.tile([HC, NV], BF16, tag="rden8")
        w_co8 = pcat.tile([128, 4, D], FP8, tag="w_co8")
        nc.sync.dma_start(out=w_co8[:], in_=r128(d_wco))
        with (
            tc.tile_pool(name="pm16", bufs=4) as pm16,
            tc.tile_pool(name="ppt", bufs=2) as ppt,
            tc.tile_pool(name="pden", bufs=2) as pden,
        ):
            with (
                tc.tile_pool(name="psim", bufs=1, space="PSUM") as psim,
                tc.tile_pool(name="pav", bufs=1, space="PSUM") as pav,
            ):
                for p in range(4):
                    avs = [pav.tile([CC + 1, NV], F32, tag=f"ps_av{e}",
                                    name=f"ps_av{e}") for e in range(2)]
                    for jc in range(NJC):
                        jw = 128 if jc < NJC - 1 else NP - 128 * (NJC - 1)
                        m8s, pss = [], []
                        for e in range(2):
                            h = 2 * p + e
                            m8 = pm16.tile([128, NV], BF16, tag=f"m8_{e}",
                                           name=f"m8_{e}")
                            nc.sync.dma_start(
                                out=m8[:jw, :],
                                in_=d_mask[h, jc * 128:jc * 128 + jw, :])
                            m8s.append(m8)
                            pss.append(psim.tile([128, NV], F32,
                                                 tag=f"ps_sim{e}",
                                                 name=f"ps_sim{e}"))
                        for (io, iw) in IC_NV:
                            for e in range(2):
                                pr = e * 64
                                nc.tensor.matmul(
                                    pss[e][:jw, io:io + iw],
                                    kT[pr:pr + 64, p, jc * 128:jc * 128 + jw],
                                    qT[pr:pr + 64, p, io:io + iw],
                                    start=True, stop=True)
                        ptms = []
                        for e in range(2):
                            pt = ppt.tile([128, NV], BF16, tag=f"pt{e}",
                                          name=f"pt{e}")
                            nc.scalar.activation(pt[:jw, :], pss[e][:jw, :],
                                                 AF.Exp, scale=SCALE)
                            ptm = ppt.tile([128, NV], BF16, tag=f"ptm{e}",
                                           name=f"ptm{e}")
                            nc.vector.tensor_mul(ptm[:jw, :], pt[:jw, :],
                                                 m8s[e][:jw, :])
                            ptms.append(ptm)
                        for (io, iw) in IC_NV:
                            for e in range(2):
                                nc.tensor.matmul(
                                    avs[e][:, io:io + iw],
                                    v1[:jw, jc, 2 * p + e, :],
                                    ptms[e][:jw, io:io + iw],
                                    start=(jc == 0), stop=(jc == NJC - 1))
                    for e in range(2):
                        pr = e * 64
                        nc.vector.tensor_copy(catR[pr:pr + 64, p, :],
                                              avs[e][0:CC, :])
                        den_st = pden.tile([1, NV], F32, tag="den_st")
                        nc.vector.tensor_copy(den_st[:], avs[e][CC:CC + 1, :])
                        nc.sync.dma_start(out=den8[2 * p + e:2 * p + e + 1, :],
                                          in_=den_st[:])
            # batched reciprocal: rden = 8/den via exp(-ln(den)+ln8) on ACT
            dln = pden.tile([HC, NV], F32, tag="dln")
            nc.scalar.activation(dln[:], den8[:], AF.Ln)
            nc.scalar.activation(rden8[:], dln[:], AF.Exp, scale=-1.0, bias=ln8_t[:])
            with tc.tile_pool(name="pdbc", bufs=2, space="PSUM") as pdbc:
                for hc in range(4):
                    for (io, iw) in IC_NV:
                        pd = pdbc.tile([128, iw], F32, tag="pd")
                        nc.tensor.matmul(pd[:], sel1[:, hc, :], rden8[:, io:io + iw],
                                         start=True, stop=True)
                        nc.vector.tensor_mul(catT[:, hc, io:io + iw],
                                             catR[:, hc, io:io + iw], pd[:])

        # Output projection (tanh/2048-folded) + residual into hsT.
        with tc.tile_pool(name="pco", bufs=2, space="PSUM") as pco:
            for mc in range(KD):
                for (io, iw) in IC_NV:
                    ps = pco.tile([128, iw], F32, tag="ps_co")
                    for kp in range(2):
                        nc.tensor.matmul(ps[:], w_co8[:, 2 * kp:2 * kp + 2, mc * 128:(mc + 1) * 128],
                                         catT[:, 2 * kp:2 * kp + 2, io:io + iw],
                                         start=(kp == 0), stop=(kp == 1),
                                         perf_mode=PM.DoubleRow)
                    nc.scalar.activation(ps[:], ps[:], AF.Identity,
                                         bias=bco_t[:, mc:mc + 1])
                    nc.vector.scalar_tensor_tensor(
                        out=hsT[mc][:, io:io + iw], in0=ps[:],
                        scalar=1.0 / (SW_CO * CAT_S),
                        in1=xT[mc][:, io:io + iw],
                        op0=ALU.mult, op1=ALU.add)
        pv1.release()
        pqk.release()
        pcat.release()
        px.release()  # xT dead

        # ================= Phase 2: LN2 + GEGLU FFN (fp8) =====================
        pln2 = tc.alloc_tile_pool(name="pln2", bufs=1)
        ln2c = [pln2.tile([128, KD, w], FP8, tag=f"ln2c{i}")
                for i, (o, w) in enumerate(IC_NV)]
        _emit_layernorm2(tc, nc, hsT, ln2c, NV, ones_r, ones_row128, eps_t, "ln2")
        penc = tc.alloc_tile_pool(name="penc", bufs=1, side="right")
        enc_sb = penc.tile([128, KT, LTP], BF16, tag="enc_sb")
        kTa = penc.tile([128, KD, LTP], BF16, tag="kTa")
        v1a = penc.tile([LTP, HA, CA + 1], BF16, tag="v1a")
        wka = penc.tile([128, KT, INNER_A], BF16, tag="wka")
        wva = penc.tile([128, KT, INNER_A], BF16, tag="wva")
        pff = tc.alloc_tile_pool(name="pff", bufs=1, side="right")
        ffT = pff.tile([128, KF, NV], FP8, tag="ffT")    # 16*(a+b)*gelu
        pwf = tc.alloc_tile_pool(name="pwf", bufs=3)
        wf0 = None
        with (
            tc.tile_pool(name="pwg", bufs=4) as pwg,
            tc.tile_pool(name="p2s", bufs=3) as p2s,
            tc.tile_pool(name="p2ps", bufs=2, space="PSUM") as p2ps,
        ):
            for m in range(KF):
                if m == 20:
                    # queue phase-3 weight DMAs mid-FFN1 so they stream
                    # behind the wg traffic instead of after it
                    nc.sync.dma_start(out=enc_sb[:], in_=r128(d_encT))
                    nc.sync.dma_start(out=v1a[:, :, CA:CA + 1],
                                      in_=d_vones.unsqueeze(2))
                    nc.sync.dma_start(out=wka[:], in_=r128(d_wka))
                    nc.sync.dma_start(out=wva[:], in_=r128(d_wva))
                if m == 30:
                    wf0 = pwf.tile([128, KF, 256], FP8, tag="wf", name="wf0")
                    nc.sync.dma_start(out=wf0[:], in_=r128(d_wf[:, 0:256]))
                if m % 2 == 0:
                    wga = pwg.tile([128, KD, 256], FP8, tag="wga")
                    nc.sync.dma_start(out=wga[:], in_=r128(d_wg[:, m * 128:(m + 2) * 128]))
                    wgg = pwg.tile([128, KD, 256], FP8, tag="wgg")
                    nc.sync.dma_start(out=wgg[:],
                                      in_=r128(d_wg[:, DFF + m * 128:DFF + (m + 2) * 128]))
                mo = (m % 2) * 128
                ps_a = p2ps.tile([128, NV], F32, tag="ps_a")
                ps_g = p2ps.tile([128, NV], F32, tag="ps_g")
                for kp in range(KD // 2):
                    for i, (io, iw) in enumerate(IC_NV):
                        nc.tensor.matmul(ps_a[:, io:io + iw], wga[:, 2 * kp:2 * kp + 2, mo:mo + 128],
                                         ln2c[i][:, 2 * kp:2 * kp + 2, :],
                                         start=(kp == 0), stop=(kp == KD // 2 - 1),
                                         perf_mode=PM.DoubleRow)
                    for i, (io, iw) in enumerate(IC_NV):
                        nc.tensor.matmul(ps_g[:, io:io + iw], wgg[:, 2 * kp:2 * kp + 2, mo:mo + 128],
                                         ln2c[i][:, 2 * kp:2 * kp + 2, :],
                                         start=(kp == 0), stop=(kp == KD // 2 - 1),
                                         perf_mode=PM.DoubleRow)
                gelu_sb = p2s.tile([128, NV], BF16, tag="gelu_sb")
                nc.scalar.activation(gelu_sb[:], ps_g[:], GELU_AF,
                                     scale=1.0 / SW_G, bias=bgg_t[:, m:m + 1])
                nc.vector.scalar_tensor_tensor(
                    out=ffT[:, m, :], in0=ps_a[:], scalar=bga_t[:, m:m + 1],
                    in1=gelu_sb[:], op0=ALU.add, op1=ALU.mult)
        # ffout (tanh/2048-folded) + residual in place.
        with (
            tc.tile_pool(name="pfps", bufs=2, space="PSUM") as pfps,
        ):
            for mc in range(KD):
                if mc == 0:
                    wf = wf0
                elif mc % 2 == 0:
                    wf = pwf.tile([128, KF, 256], FP8, tag="wf")
                    nc.sync.dma_start(out=wf[:], in_=r128(d_wf[:, mc * 128:(mc + 2) * 128]))
                mo = (mc % 2) * 128
                pss = [pfps.tile([128, iw], F32, tag=f"ps_f{i}")
                       for i, (io, iw) in enumerate(IC_NV)]
                for kp in range(KF // 2):
                    for i, (io, iw) in enumerate(IC_NV):
                        nc.tensor.matmul(pss[i][:], wf[:, 2 * kp:2 * kp + 2, mo:mo + 128],
                                         ffT[:, 2 * kp:2 * kp + 2, io:io + iw],
                                         start=(kp == 0), stop=(kp == KF // 2 - 1),
                                         perf_mode=PM.DoubleRow)
                for i, (io, iw) in enumerate(IC_NV):
                    nc.scalar.activation(pss[i][:], pss[i][:], AF.Identity,
                                         bias=bf_t[:, mc:mc + 1])
                    nc.vector.scalar_tensor_tensor(
                        out=hsT[mc][:, io:io + iw], in0=pss[i][:],
                        scalar=1.0 / (SW_F * SW_G),
                        in1=hsT[mc][:, io:io + iw],
                        op0=ALU.mult, op1=ALU.add)
        pwf.release()
        pln2.release()
        pff.release()

        # ============== Phase 3: enc projections, LN3, q3, cross-attn =========
        # enc k/v projections are independent of LN3 (weights were DMA'd
        # during FFN1) — emit first so they overlap the LN3 stats.
        with tc.tile_pool(name="ppenc", bufs=2, space="PSUM") as ppenc:
            for mc in range(KD):
                ps = ppenc.tile([128, LTP], F32, tag="ps_enc")
                for kc in range(KT):
                    nc.tensor.matmul(ps[:], wka[:, kc, mc * 128:(mc + 1) * 128],
                                     enc_sb[:, kc, :], start=(kc == 0), stop=(kc == KT - 1))
                nc.vector.tensor_copy(kTa[:, mc, :], ps[:])
            for (co, cw) in DC_D:
                ps = ppenc.tile([LTP, 512], F32, tag="ps_encv")
                for kc in range(KT):
                    nc.tensor.matmul(ps[:, :cw], enc_sb[:, kc, :], wva[:, kc, co:co + cw],
                                     start=(kc == 0), stop=(kc == KT - 1))
                nc.vector.tensor_copy(v1a[:, co // CA:(co + cw) // CA, 0:CA],
                                      ps[:, :cw].rearrange("p (h c) -> p h c", c=CA))

        pln3 = tc.alloc_tile_pool(name="pln3", bufs=1, side="right")
        ln3c = [pln3.tile([128, KD, w], FP8, tag=f"ln3c{i}")
                for i, (o, w) in enumerate(IC_NV)]
        _emit_layernorm2(tc, nc, hsT, ln3c, NV, ones_r, ones_row128, eps_t, "ln3")
        res.release()  # hsT dead

        pq3 = tc.alloc_tile_pool(name="pq3", bufs=1)
        qTa = pq3.tile([128, KD, NV], BF16, tag="qTa")
        with (
            tc.tile_pool(name="pwqa", bufs=3) as pwqa,
            tc.tile_pool(name="pp3", bufs=2, space="PSUM") as pp3,
        ):
            for half in range(5):
                wqa = pwqa.tile([128, KD, 256], FP8, tag="wqa")
                nc.sync.dma_start(out=wqa[:],
                                  in_=r128(d_wqa[:, half * 256:(half + 1) * 256]))
                for mh in range(2):
                    mc = half * 2 + mh
                    pss = [pp3.tile([128, iw], F32, tag=f"ps_p3{i}")
                           for i, (io, iw) in enumerate(IC_NV)]
                    for kp in range(KD // 2):
                        for i, (io, iw) in enumerate(IC_NV):
                            nc.tensor.matmul(pss[i][:], wqa[:, 2 * kp:2 * kp + 2, mh * 128:(mh + 1) * 128],
                                             ln3c[i][:, 2 * kp:2 * kp + 2, :],
                                             start=(kp == 0), stop=(kp == KD // 2 - 1),
                                             perf_mode=PM.DoubleRow)
                    for i, (io, iw) in enumerate(IC_NV):
                        nc.vector.tensor_scalar(qTa[:, mc, io:io + iw], pss[i][:],
                                                1.0 / SW_QKV, qab_t[:, mc:mc + 1],
                                                ALU.mult, ALU.add)
        pln3.release()

        # cross-attention with 2-group denominator + 2-pass output projection
        pcat3 = tc.alloc_tile_pool(name="pcat3", bufs=1, side="right")
        catTa = [pcat3.tile([128, NV], BF16, tag=f"catTa{hc}") for hc in range(KD)]
        catB = [pcat3.tile([128, NV], BF16, tag=f"catB{hc}") for hc in range(KD)]
        den_g = [pcat3.tile([KD, NV], F32, tag=f"den_g{g}") for g in range(2)]
        rden_g = [pcat3.tile([KD, NV], BF16, tag=f"rden_g{g}") for g in range(2)]
        popart = tc.alloc_tile_pool(name="popart", bufs=1)
        opart = [popart.tile([128, D], F32, tag=f"opart{it}") for it in range(8)]
        pwoa = tc.alloc_tile_pool(name="pwoa", bufs=1)
        woa = pwoa.tile([128, KD, D], BF16, tag="woa")
        nc.sync.dma_start(out=woa[:], in_=r128(d_woa))

        with (
            tc.tile_pool(name="p3s", bufs=3) as p3s,
            tc.tile_pool(name="pden3", bufs=2) as pden3,
            tc.tile_pool(name="psa", bufs=1, space="PSUM") as psa,
            tc.tile_pool(name="pava", bufs=1, space="PSUM") as pava,
            tc.tile_pool(name="pdbca", bufs=1, space="PSUM") as pdbca,
            tc.tile_pool(name="poo", bufs=1, space="PSUM") as poo,
            tc.tile_pool(name="po", bufs=3) as po,
        ):
            ptas = {}

            def emit_sim3(h):
                pr = (h % 2) * 64
                hc = h // 2
                pta = p3s.tile([LTP, NV], BF16, tag=f"pta{h % 3}")
                for i, (io, iw) in enumerate(IC_NV):
                    ps_s = psa.tile([LTP, 512], F32, tag=f"ps_sa{i}")
                    nc.tensor.matmul(ps_s[:, :iw], kTa[pr:pr + 64, hc, :],
                                     qTa[pr:pr + 64, hc, io:io + iw],
                                     start=True, stop=True)
                    nc.scalar.activation(pta[:, io:io + iw], ps_s[:, :iw], AF.Exp,
                                         scale=SCALE)
                ptas[h] = pta

            def emit_av3(h):
                pr = (h % 2) * 64
                hc = h // 2
                g = h // 10
                pta = ptas.pop(h)
                av = pava.tile([CA + 1, NV], F32, tag="ps_ava")
                for (io, iw) in IC_NV:
                    nc.tensor.matmul(av[:, io:io + iw], v1a[:, h, :],
                                     pta[:, io:io + iw], start=True, stop=True)
                if h % 2 == 0:
                    nc.vector.tensor_copy(catTa[hc][0:64, :], av[0:CA, :])
                else:
                    nc.scalar.activation(catTa[hc][64:128, :], av[0:CA, :], AF.Copy)
                den_st = pden3.tile([1, NV], F32, tag="den_st3")
                nc.vector.tensor_copy(den_st[:], av[CA:CA + 1, :])
                nc.sync.dma_start(out=den_g[g][h - 10 * g:h - 10 * g + 1, :],
                                  in_=den_st[:])

            def emit_group_norm(g):
                # rden for heads 10g..10g+9, then catB for hc 5g..5g+4;
                # sel3g row index is the within-group head index.
                dln = pden3.tile([KD, NV], F32, tag=f"dln3{g}")
                nc.scalar.activation(dln[:], den_g[g][:], AF.Ln)
                nc.scalar.activation(rden_g[g][:], dln[:], AF.Exp, scale=-1.0)
                for hc in range(5 * g, 5 * g + 5):
                    for (io, iw) in IC_NV:
                        pd = pdbca.tile([128, iw], F32, tag="pda")
                        nc.tensor.matmul(pd[:], sel3g[:, hc - 5 * g, :],
                                         rden_g[g][:, io:io + iw],
                                         start=True, stop=True)
                        nc.vector.tensor_mul(catB[hc][:, io:io + iw],
                                             catTa[hc][:, io:io + iw], pd[:])

            def emit_oproj_pass(kcs, first):
                for it in range(NV // 128):
                    for i, (dco, dcw) in enumerate(DC_D):
                        ps = poo.tile([128, dcw], F32, tag=f"ps_oo{i}")
                        for j, kc in enumerate(kcs):
                            nc.tensor.matmul(ps[:], catB[kc][:, it * 128:(it + 1) * 128],
                                             woa[:, kc, dco:dco + dcw],
                                             start=(j == 0), stop=(j == len(kcs) - 1))
                        if first:
                            nc.vector.tensor_add(opart[it][:, dco:dco + dcw], ps[:],
                                                 boa_b[:, dco:dco + dcw])
                        else:
                            o_sb = po.tile([128, dcw], F32, tag="o_sb")
                            nc.vector.tensor_add(o_sb[:], ps[:],
                                                 opart[it][:, dco:dco + dcw])
                            nc.sync.dma_start(
                                out=d_out[it * 128:(it + 1) * 128, dco:dco + dcw],
                                in_=o_sb[:])

            sel3g = cst.tile([KD, 5, 128], BF16, tag="sel3g")
            nc.vector.tensor_copy(sel3g[:], sel3[0:KD, 0:5, :])

            emit_sim3(0)
            emit_sim3(1)
            for h in range(2, 10):
                emit_sim3(h)
                emit_av3(h - 2)
            emit_av3(8)
            emit_av3(9)
            emit_group_norm(0)
            emit_sim3(10)
            emit_sim3(11)
            emit_oproj_pass(list(range(5)), True)
            for h in range(12, 20):
                emit_sim3(h)
                emit_av3(h - 2)
            emit_av3(18)
            emit_av3(19)
            emit_group_norm(1)
            emit_oproj_pass(list(range(5, KD)), False)
        pcat3.release()
        pwoa.release()
        popart.release()
        pq3.release()
        penc.release()
        cst.release()

    nc.compile()
    return nc


_CACHE = {}


def _get_nc():
    if "nc" not in _CACHE:
        _CACHE["nc"] = build_nc()
    return _CACHE["nc"]


def _sel(nh, npairs):
    bf16 = ml_dtypes.bfloat16
    s = np.zeros((nh, npairs, 128), dtype=bf16)
    for hc in range(npairs):
        s[2 * hc, hc, 0:64] = 1
        s[2 * hc + 1, hc, 64:128] = 1
    return s


def prepare_in_maps(inputs):
    f32 = np.float32
    bf16 = ml_dtypes.bfloat16
    fp8 = ml_dtypes.float8_e4m3
    hidT = np.ascontiguousarray(inputs["hidden_states"].transpose(0, 2, 1)).astype(bf16)
    objT = np.ascontiguousarray(inputs["object_embeddings"].transpose(0, 2, 1)).astype(bf16)
    encT = np.zeros((B, DTXT, LTP), dtype=f32)
    encT[:, :, :LTXT] = inputs["encoder_hidden_states"].transpose(0, 2, 1)
    masks = inputs["object_attention_masks"]
    mask8 = np.zeros((B, HC, NP, NV), dtype=bf16)
    mask8[:, :, :N, :] = (masks.transpose(0, 1, 3, 2)[:, :, :, :NV] > 0)

    ta = float(np.tanh(inputs["alpha_attn"]))
    td = float(np.tanh(inputs["alpha_dense"]))
    g1 = np.asarray(inputs["norm1_g"], dtype=f32)
    b1 = np.asarray(inputs["norm1_b"], dtype=f32)
    g2 = np.asarray(inputs["norm2_g"], dtype=f32)
    b2 = np.asarray(inputs["norm2_b"], dtype=f32)
    g3 = np.asarray(inputs["norm3_g"], dtype=f32)
    b3 = np.asarray(inputs["norm3_b"], dtype=f32)

    w_q = np.asarray(inputs["cma_q_w"], dtype=f32)
    w_k = np.asarray(inputs["cma_k_w"], dtype=f32)
    w_v = np.asarray(inputs["cma_v_w"], dtype=f32)
    w_co = np.asarray(inputs["cma_out_w"], dtype=f32)
    # beta folds: q bias explicit; k bias cancels in softmax; v bias shifts
    # every attention output by vb (softmax weights sum to 1) -> co bias.
    qb = b1 @ w_q                       # [512]
    vb = b1 @ w_v                       # [512]
    bco = np.asarray(inputs["cma_out_b"], dtype=f32) + vb @ w_co
    w_geglu = np.asarray(inputs["geglu_w"], dtype=f32)
    bg = np.asarray(inputs["geglu_b"], dtype=f32) + b2 @ w_geglu
    w_qa = np.asarray(inputs["attn_q_w"], dtype=f32)
    qab = b3 @ w_qa                     # [1280]

    w_co8 = (w_co * (ta * SW_CO)).astype(fp8)
    bco_cols = np.ascontiguousarray(
        (bco * (ta * SW_CO * CAT_S)).reshape(KD, 128).T)
    w_f8 = (np.asarray(inputs["ffout_w"]) * (td * SW_F)).astype(fp8)
    bf_cols = np.ascontiguousarray(
        (np.asarray(inputs["ffout_b"], dtype=f32) * (td * SW_F * SW_G))
        .reshape(KD, 128).T)
    shared = {
        "w_lin": np.asarray(inputs["linear_w"]).astype(bf16),
        "b_lin": np.ascontiguousarray(inputs["linear_b"], dtype=f32),
        "w_q8": (w_q * g1[:, None] * SW_QKV).astype(fp8),
        "w_k8": (w_k * g1[:, None] * SW_QKV).astype(fp8),
        "w_v8": (w_v * g1[:, None] * SW_QKV).astype(fp8),
        "qb_cols": np.ascontiguousarray(qb.reshape(4, 128).T, dtype=f32),
        "w_co8": w_co8, "bco_cols": bco_cols,
        "w_g8": (w_geglu * g2[:, None] * SW_G).astype(fp8),
        "bg_a16": (bg[:DFF] * SW_G).astype(f32),
        "bg_g": bg[DFF:].astype(f32),
        "w_f8": w_f8, "bf_cols": bf_cols,
        "w_qa8": (w_qa * g3[:, None]).astype(bf16),
        "qab_cols": np.ascontiguousarray(qab.reshape(KD, 128).T, dtype=f32),
        "w_ka": np.asarray(inputs["attn_k_w"]).astype(bf16),
        "w_va": np.asarray(inputs["attn_v_w"]).astype(bf16),
        "w_oa": np.asarray(inputs["attn_out_w"]).astype(bf16),
        "b_oa": np.ascontiguousarray(inputs["attn_out_b"], dtype=f32),
        "vones": np.concatenate([np.ones((LTXT, HA)),
                                 np.zeros((LTP - LTXT, HA))], axis=0).astype(bf16),
        "sel1": _sel(HC, 4), "sel3": _sel(HA, KD),
    }
    in_maps = []
    for b in range(B):
        m = dict(shared)
        m["hidT"] = hidT[b]
        m["objT"] = objT[b]
        m["encT"] = encT[b].astype(bf16)
        m["mask8"] = np.ascontiguousarray(mask8[b])
        in_maps.append(m)
    return in_maps


def run(inputs, trace=False):
    nc = _get_nc()
    in_maps = prepare_in_maps(inputs)
    res = run_bass_kernel_spmd(nc, in_maps, core_ids=list(range(B)), trace=trace)
    out = np.stack([res.results[b]["out"] for b in range(B)], axis=0)
    return out, res


def kernel(**inputs):
    out, _ = run(inputs, trace=False)
    return out
